# revision 44
# baseline (speedup 1.0000x reference)
"""AttentionNeuronLayer Trainium2 kernel.

Strategy: the obs_dim rows of the LSTM state evolve fully independently
through the whole recurrence (each obs scalar is paired with the full
action vector); only the final `out = w @ s` contracts over obs rows, and
the final tanh is applied after that sum.  So we shard obs_dim 512 -> 64
rows per core across 8 cores with zero collectives: each core runs the
full T=256 recurrence on its 64 rows in a transposed layout
(pos_em/gate/msg dims on partitions, obs rows on the free dim), computes
its slice of w = tanh(q @ k.T / sqrt(d)) and the partial contraction
w.T @ s, and the host sums the 8 partial (T, hidden) results and applies
the final tanh.

The per-step critical path is latency-bound: one serial chain
PE(whh) -> Act(sigma) -> elementwise c-update -> Act(tanh c) ->
h16-mul -> PE per step, each cross-engine hop costing ~100ns of
semaphore visibility, and the Act engine carries the attention tail's
tanh(w) on top of its two chain ops.  In steady state every Act gap is
filled and the step settles at ~1.43us = sigma(398) + w-chunk(412) +
tct(238) + w-chunk(385), Act-saturated.  Scheduling choices:
  * the x/a input half of the gates is accumulated into the step's PSUM
    bank ahead of time as one accumulation group (first xa matmul opens
    it, last whh matmul closes it), so after h16 lands only the four
    27ns W_hh matmuls remain on the chain;
  * one merged 256-wide sigmoid covers i,f,2g,o (g doubled so
    tanh(g) = 2*sigmoid(2g)-1 folds into the c-update);
  * the whole c-update (u = 2*sig2g-1, m = u*i, c = f*c + m) and
    h16 = sigma_o * tct run on the otherwise-idle GPSIMD engine, whose
    elementwise ops carry no SBUF-access bubble; sigma outputs, t1, cT,
    and tct are fp16;
  * the attention tail is pipelined behind the recurrence: fp16-h kT
    matmuls (hi/lo Wk weights), k-bias folded into the DVE PSUM->SBUF
    cast, w matmuls (the second hidden-half scheduling-pinned behind the
    next whh so they never head-of-line block the chain on the in-order
    PE), and tanh(w) split into 272/240-wide chunks pinned into the two
    Act gaps of each step (after sigma and after tct);
  * poT out-accumulation flushes in 128-col quarters through the DVE's
    idle window; input DMAs are chunked and spread across queues so the
    recurrence starts ~3.5us in; a dummy-matmul burst pre-warms the PE
    clock ramp during the DMA wait.
"""

import sys

sys.path.insert(0, "/opt/trn_rl_repo")

import numpy as np

import bass_rust
import concourse.bass as bass
import concourse.tile as tile
from concourse import mybir
from concourse.vector_clock import ScopedClock
from concourse.bass_utils import run_bass_kernel_spmd

OBS_DIM = 512
ACT_DIM = 32
HIDDEN_DIM = 1024
MSG_DIM = 128
POS_EM_DIM = 128
T = 256
NCORES = 8
SH = OBS_DIM // NCORES  # 64 obs rows per core

F32 = mybir.dt.float32
F16 = mybir.dt.float16
AF = mybir.ActivationFunctionType
ALU = mybir.AluOpType

# gate blocks laid out [i, f, g, o]; g row pre-scaled by 2
_PERM = [0, 1, 2, 3]

TRACE = [False]  # test.py flips this for the profiled run
LAST_RESULTS = [None]
INPUT_NAMES = ("whhT", "wih3T", "wkThi", "wkTlo", "bkc", "qT", "xa3", "xTp")


def _patched_drain_and_barrier(self, tick_clock, wait_clock):
    # This walrus build rejects instructions carrying more than one
    # sync-wait command; Tile's tail drain aggregates one wait per live
    # proc.  Re-emit the waits on individual single-wait NOPs instead.
    nc = self.nc
    carrier = nc.sync.nop(nofuse=True)
    wait_clock.add_sem_waits(carrier.ins, ScopedClock({None: tick_clock.global_clock}))
    si = carrier.ins.sync_info
    waits = list(si.on_wait) if si is not None and si.on_wait else []
    if si is not None:
        carrier.ins.sync_info = mybir.SyncInfo(
            on_wait=[], on_update=list(si.on_update or [])
        )
    for w in waits:
        n2 = nc.sync.nop(nofuse=True)
        n2.ins.sync_info = mybir.SyncInfo(on_wait=[w], on_update=[])
    nc.sync.drain()
    nc.all_engine_barrier()
    popped = nc._tile_sem_poison_stack.pop()
    assert popped is self._sem_poison
    nc.clear_and_free_semaphores(list(self.sems.allocated().values()))
    nc.all_engine_barrier()


tile.TileContext._drain_and_barrier = _patched_drain_and_barrier


def _split_multi_waits(module):
    """This walrus build accepts at most one sync-wait command per
    instruction.  Move excess waits onto same-engine NoOps inserted just
    before the instruction — the engine stream is serial, so gating an
    earlier NoOp on the same conditions is equivalent (DMA triggers are
    issued by their engine in program order, so this holds for DMACopy
    too)."""
    import copy as _copy

    counter = [0]
    new_module = _copy.replace(module, functions=[])
    for function in module.functions:
        new_function = _copy.replace(function, blocks=[])
        new_function.set_allocations_from_list(function.allocations)
        for block in function.blocks:
            new_insts = []
            for inst in block.instructions:
                si = inst.sync_info
                waits = list(si.on_wait) if si is not None and si.on_wait else []
                if len(waits) > 1:
                    for w in waits[:-1]:
                        counter[0] += 1
                        nop = mybir.InstNoOp(
                            engine=inst.engine, name=f"I-ws{counter[0]}"
                        )
                        nop.sync_info = mybir.SyncInfo(on_wait=[w], on_update=[])
                        new_insts.append(nop)
                    inst.sync_info = mybir.SyncInfo(
                        on_wait=[waits[-1]], on_update=list(si.on_update or [])
                    )
                new_insts.append(inst)
            new_function.blocks.append(_copy.replace(block, instructions=new_insts))
        new_module.functions.append(new_function)
    return new_module


_NC_CACHE = {}


def _build_nc(split=True):
    if split in _NC_CACHE:
        return _NC_CACHE[split]
    nc = bass.Bass()
    whhT = nc.declare_dram_parameter("whhT", [POS_EM_DIM, 512], F16, isOutput=False)
    wih3T = nc.declare_dram_parameter("wih3T", [102, 512], F16, isOutput=False)
    wkThi = nc.declare_dram_parameter("wkThi", [POS_EM_DIM, MSG_DIM], F16, isOutput=False)
    wkTlo = nc.declare_dram_parameter("wkTlo", [POS_EM_DIM, MSG_DIM], F16, isOutput=False)
    bkc = nc.declare_dram_parameter("bkc", [MSG_DIM, 1], F32, isOutput=False)
    qT = nc.declare_dram_parameter("qT", [MSG_DIM, HIDDEN_DIM], F16, isOutput=False)
    xa3 = nc.declare_dram_parameter("xa3", [102, T * SH], F16, isOutput=False)
    xTp = nc.declare_dram_parameter("xTp", [2 * SH, T], F16, isOutput=False)
    # transposed out accumulation: 64 steps (32 pairs x 8 hidden blocks x
    # N=2) fill one (128, 512) PSUM bank; host decodes the layout
    outs = nc.declare_dram_parameter("outs", [T // 64, 128, 512], F32, isOutput=True)

    inv_scale = 1.0 / float(np.sqrt(np.float32(MSG_DIM)))

    with tile.TileContext(nc) as tc:
        with (
            tc.tile_pool(name="const", bufs=1) as const,
            tc.tile_pool(name="state", bufs=1) as state,
            tc.tile_pool(name="hs", bufs=4) as hsp,
            tc.tile_pool(name="work", bufs=3) as work,
            tc.tile_pool(name="wap", bufs=2) as wap,
            tc.tile_pool(name="stg", bufs=2) as stg,
            tc.tile_pool(name="pg", bufs=2, space="PSUM") as pgp,
            tc.tile_pool(name="pk", bufs=1, space="PSUM") as pkp,
            tc.tile_pool(name="pw", bufs=2, space="PSUM") as pwp,
            tc.tile_pool(name="po", bufs=1, space="PSUM") as pop,
        ):
            whhT_sb = const.tile([POS_EM_DIM, 512], F16)
            wih3T_sb = const.tile([102, 512], F16)
            wkThi_sb = const.tile([POS_EM_DIM, MSG_DIM], F16)
            wkTlo_sb = const.tile([POS_EM_DIM, MSG_DIM], F16)
            bkc_sb = const.tile([MSG_DIM, 1], F32)
            qT_sb = const.tile([MSG_DIM, HIDDEN_DIM], F16)
            xa3_sb = const.tile([102, T * SH], F16)
            xTp_sb = const.tile([2 * SH, T], F16)
            # the recurrence can start once the weights and the first slice
            # of xa3 are resident: front-load those on the SP queue and
            # spread the rest (xa3 streams far ahead of consumption) across
            # the other engines' DMA queues so nothing serializes behind
            # the 3.2MB xa3 transfer
            XQ = T * SH // 4
            # all bulk DMAs ride the SP queue (SP is otherwise idle until
            # the first out-flush at ~100us); only qT uses the Act queue
            # (lands well before the first w matmuls).  Pool/DVE must stay
            # clear: DMA transfers occupy the issuing engine in the cost
            # model, and the chain needs them from ~7us on.
            nc.sync.dma_start(out=whhT_sb[:], in_=whhT[:])
            nc.sync.dma_start(out=wih3T_sb[:], in_=wih3T[:])
            nc.sync.dma_start(out=xa3_sb[:, 0 : XQ // 2], in_=xa3[:, 0 : XQ // 2])
            nc.scalar.dma_start(out=qT_sb[:], in_=qT[:])
            nc.sync.dma_start(out=xa3_sb[:, XQ // 2 : XQ], in_=xa3[:, XQ // 2 : XQ])
            nc.sync.dma_start(out=wkThi_sb[:], in_=wkThi[:])
            nc.sync.dma_start(out=wkTlo_sb[:], in_=wkTlo[:])
            nc.sync.dma_start(out=bkc_sb[:], in_=bkc[:])
            nc.sync.dma_start(out=xTp_sb[:], in_=xTp[:])
            nc.sync.dma_start(out=xa3_sb[:, XQ : 2 * XQ], in_=xa3[:, XQ : 2 * XQ])
            nc.sync.dma_start(
                out=xa3_sb[:, 2 * XQ : 3 * XQ], in_=xa3[:, 2 * XQ : 3 * XQ]
            )
            nc.sync.dma_start(
                out=xa3_sb[:, 3 * XQ : 4 * XQ], in_=xa3[:, 3 * XQ : 4 * XQ]
            )

            cT = state.tile([POS_EM_DIM, SH], F16)
            h0 = const.tile([POS_EM_DIM, SH], F16)
            nc.vector.memset(cT[:], 0.0)
            nc.vector.memset(h0[:], 0.0)
            # warm the sigmoid/tanh act table while the input DMAs stream
            scr = const.tile([128, 1], F32)
            nc.scalar.activation(scr[:], h0[:, 0:1], AF.Sigmoid)

            poT = pop.tile([128, 512], F32)
            nc.vector.memset(poT[:], 0.0)

            # PE p-state pre-warm: ~3us of dummy matmuls while the input
            # DMAs stream, so the real recurrence starts at full clock
            # (the tensor engine ramps 0.65 -> 2.4 GHz over 3us of use)
            pwm = pkp.tile([MSG_DIM, 2 * SH], F32, name="pwm", tag="pk")
            for _ in range(60):
                nc.tensor.matmul(pwm[0:64, 0:64], h0[:, 0:64], h0[:, 0:64], start=True, stop=True)

            h16bufs = {}
            kbufs = {}
            wabufs = {}
            pgbufs = {}
            pending_flush = []
            whh_last = [None]
            so_cur = [None]

            def emit_xa(t):
                # x/a input half of step t's gates: no h dependence, runs in
                # PE idle time ahead of the chain.  One PSUM accumulation
                # group covers the whole step (zero regions are
                # bank-granular, so only one group may be open per bank):
                # the first xa matmul opens it, the last whh matmul closes
                # it.  Full-bank tile so nothing else shares the region.
                pg = pgp.tile([128, 512], F32, name=f"pg{t}", tag="pg")[:, 0:256]
                pgbufs[t] = pg
                for b in range(4):
                    nc.tensor.matmul(
                        pg[:, 64 * b : 64 * b + 64],
                        wih3T_sb[:, 128 * b : 128 * b + 128],
                        xa3_sb[:, SH * t : SH * t + SH],
                        start=(b == 0),
                        stop=False,
                    )

            def emit_kT(p):
                # k = Wk @ h for the step pair, fp16 h with hi/lo weights
                pk = pkp.tile([MSG_DIM, 2 * SH], F32, tag="pk")
                nc.tensor.matmul(
                    pk[:], wkThi_sb[:], h16bufs[p][:], start=True, stop=False
                )
                nc.tensor.matmul(
                    pk[:], wkTlo_sb[:], h16bufs[p][:], start=False, stop=True
                )
                # PSUM->SBUF fp16 cast with the k-bias folded in; GPSIMD
                # cannot read PSUM, so this rides the DVE's idle window
                # between the c-chain and the next step
                kTp = work.tile([MSG_DIM, 2 * SH], F16, name=f"kTp{p}", tag="kTp")
                nc.vector.tensor_scalar_add(kTp[:], pk[:], bkc_sb[:])
                kbufs[p] = kTp

            def emit_w_mms(p):
                # rows 0-63 = even step of the pair, 64-127 = odd step.
                # The second hidden-half's matmuls get scheduling edges
                # behind the NEXT step's whh matmuls (pending_wdep) so the
                # 213ns w matmuls never head-of-line block the chain on
                # the in-order PE.
                kTp = kbufs.pop(p)
                pw = pwp.tile([128, HIDDEN_DIM], F32, name=f"pw{p}", tag="pw")
                for h2 in range(2):
                    for jj in range(2):
                        tp = None if jj == 0 else (0, 64)
                        mm = nc.tensor.matmul(
                            pw[64 * jj : 64 * jj + 64, 512 * h2 : 512 * h2 + 512],
                            kTp[:, SH * jj : SH * jj + SH],
                            qT_sb[:, 512 * h2 : 512 * h2 + 512],
                            start=True,
                            stop=True,
                            tile_position=tp,
                        )
                        if h2 == 1:
                            bass_rust.add_dep_helper(
                                mm.ins,
                                whh_last[0].ins,
                                sync=False,
                                reason="w mm after whh",
                            )
                wa = wap.tile([128, HIDDEN_DIM], F16, name=f"wa{p}", tag="wa")
                wabufs[p] = pw, wa

            def emit_wa_chunk(p, lo, hi, after=None, final=False):
                # tanh over one slice of w, sized to ride an Act gap of the
                # chain (320 cols behind sigma, 192 behind tct).  The
                # scheduler orders by readiness, so pin it behind the chain
                # op with a no-sync edge (same engine, program order is
                # enough).
                pw, wa = wabufs[p]
                ch = nc.scalar.activation(
                    wa[:, lo:hi], pw[:, lo:hi], AF.Tanh, scale=inv_scale
                )
                if after is not None:
                    bass_rust.add_dep_helper(
                        ch.ins, after.ins, sync=False, reason="wa chunk placement"
                    )
                if final:
                    wabufs[p] = wa

            def emit_out(p):
                # transposed: out rows = hidden sub-dim (M=128), col pair =
                # the two steps; 32 pairs accumulate in one PSUM bank
                wa = wabufs.pop(p)
                base = 16 * (p % 32)
                for i in range(8):
                    nc.tensor.matmul(
                        poT[:, base + 2 * i : base + 2 * i + 2],
                        wa[:, 128 * i : 128 * i + 128],
                        xTp_sb[:, 2 * p : 2 * p + 2],
                        start=True,
                        stop=True,
                    )
                if p % 8 == 7:
                    pending_flush.append((p // 32, (p % 32) // 8, None))

            emit_xa(0)
            emit_xa(1)
            for t in range(T):
                j = t % 2  # position within the step pair
                p = t // 2
                if j == 0:
                    h16bufs[p] = hsp.tile(
                        [POS_EM_DIM, 2 * SH], F16, name=f"h16p{p}", tag="h16p"
                    )
                h_prev = (
                    h0[:]
                    if t == 0
                    else (
                        h16bufs[p - 1][:, SH:] if j == 0 else h16bufs[p][:, 0:SH]
                    )
                )
                # ---- gates: W_hh half, on the chain ----
                pg = pgbufs.pop(t)
                for b in range(4):
                    wm = nc.tensor.matmul(
                        pg[:, 64 * b : 64 * b + 64],
                        whhT_sb[:, 128 * b : 128 * b + 128],
                        h_prev,
                        start=False,
                        stop=(b == 3),
                    )
                whh_last[0] = wm
                # ---- merged sigmoid over i,f,2g,o (fp16 out) ----
                sig = work.tile([128, 256], F16, tag="sig")
                sg = nc.scalar.activation(sig[:], pg[:], AF.Sigmoid)
                # attention-tail tanh chunk rides the Act gap here
                if p >= 2:
                    emit_wa_chunk(p - 2, 512 * j, 512 * j + 272, after=sg)
                # ---- c ----  (tanh(g) = 2*sigmoid(2g) - 1)
                t1 = work.tile([128, SH], F16, tag="t1")
                nc.gpsimd.tensor_scalar(
                    t1[:], sig[:, 128:192], 2.0, -1.0, ALU.mult, ALU.add
                )
                nc.gpsimd.tensor_mul(t1[:], t1[:], sig[:, 0:64])
                nc.gpsimd.tensor_mul(cT[:], sig[:, 64:128], cT[:])
                ca = nc.gpsimd.tensor_add(cT[:], cT[:], t1[:])
                if pending_flush:
                    g, hh, _ = pending_flush.pop(0)
                    if hh == 0:
                        so_cur[0] = stg.tile([128, 512], F32, name="so", tag="so")
                    so = so_cur[0]
                    cp = nc.vector.tensor_copy(
                        so[:, 128 * hh : 128 * hh + 128],
                        poT[:, 128 * hh : 128 * hh + 128],
                    )
                    bass_rust.add_dep_helper(
                        cp.ins, ca.ins, sync=False, reason="flush placement"
                    )
                    if hh == 3:
                        nc.sync.dma_start(
                            out=outs[g, :, 384:512], in_=so[:, 384:512]
                        )
                    elif hh == 2:
                        nc.sync.dma_start(
                            out=outs[g, :, 0:384], in_=so[:, 0:384]
                        )
                tct = work.tile([128, SH], F16, tag="tct")
                tc = nc.scalar.activation(tct[:], cT[:], AF.Tanh)
                if p >= 2:
                    emit_wa_chunk(
                        p - 2,
                        512 * j + 272,
                        512 * j + 512,
                        after=tc,
                        final=(j == 1),
                    )
                # ---- h16 = sigma_o * tanh(c), on GPSIMD ----
                nc.gpsimd.tensor_mul(
                    h16bufs[p][:, SH * j : SH * j + SH], sig[:, 192:256], tct[:]
                )
                # ---- pipelined attention tail + next xa prefetch ----
                if j == 1:
                    if p >= 1:
                        emit_kT(p - 1)
                        emit_w_mms(p - 1)
                    if p == T // 2 - 1:
                        emit_kT(p)
                        emit_w_mms(p)
                    if p >= 2:
                        emit_out(p - 2)
                    h16bufs.pop(p - 2, None)
                if t + 2 < T:
                    emit_xa(t + 2)
            last = T // 2 - 1  # 127
            for q in (last - 1, last):
                emit_wa_chunk(q, 0, 512)
                emit_wa_chunk(q, 512, 1024, final=True)
                emit_out(q)
            if pending_flush:
                g, hh, _ = pending_flush[0]
                if hh == 0:
                    so_cur[0] = stg.tile([128, 512], F32, name="so", tag="so")
                so = so_cur[0]
                nc.vector.tensor_copy(so[:, 128 * hh : 512], poT[:, 128 * hh : 512])
                if hh <= 2:
                    nc.sync.dma_start(out=outs[g, :, 0:384], in_=so[:, 0:384])
                nc.sync.dma_start(out=outs[g, :, 384:512], in_=so[:, 384:512])
    if split:
        nc.m = _split_multi_waits(nc.m)
    _NC_CACHE[split] = nc
    return nc


def kernel(
    obs,
    prev_act,
    in_shift,
    in_scale,
    pos_embedding,
    W_ih,
    b_ih,
    W_hh,
    b_hh,
    Wq,
    bq,
    Wk,
    bk,
):
    obs = np.asarray(obs, np.float32)
    prev_act = np.asarray(prev_act, np.float32)
    in_shift = np.asarray(in_shift, np.float32)
    in_scale = np.asarray(in_scale, np.float32)
    pos_embedding = np.asarray(pos_embedding, np.float32)
    W_ih = np.asarray(W_ih, np.float32)
    b_ih = np.asarray(b_ih, np.float32)
    W_hh = np.asarray(W_hh, np.float32)
    b_hh = np.asarray(b_hh, np.float32)
    Wq = np.asarray(Wq, np.float32)
    bq = np.asarray(bq, np.float32)
    Wk = np.asarray(Wk, np.float32)
    bk = np.asarray(bk, np.float32)

    x = (obs - in_shift) / (in_scale + 1e-8)  # (T, 512)
    q = pos_embedding @ Wq.T + bq  # (1024, 128)
    qT = np.ascontiguousarray(q.T)  # (128, 1024)

    def blocks(mat_rows):  # reorder gate blocks to [i, f, g, o]
        return np.concatenate([mat_rows[128 * p : 128 * p + 128] for p in _PERM], 0)

    W_ih_r = blocks(W_ih)  # (512, 33)
    W_hh_r = blocks(W_hh)  # (512, 128)
    b_r = blocks((b_ih + b_hh)[:, None])[:, 0]  # (512,)
    # g block (cols 256:384 after reorder) doubled: tanh(g) = 2*sig(2g)-1
    gs = np.ones((512, 1), np.float32)
    gs[256:384] = 2.0
    W_ih_r = W_ih_r * gs
    W_hh_r = W_hh_r * gs
    b_r = b_r * gs[:, 0]

    whhT = np.ascontiguousarray(W_hh_r.T)  # (128, 512)
    wih1T = np.concatenate(
        [W_ih_r[:, 0:1].T, np.ascontiguousarray(W_ih_r[:, 1:33].T), b_r[None, :]], 0
    )  # (34, 512)
    # fp16 hi/lo folded into K: [Whi; Whi; Wlo] x [xhi; xlo; xhi] gives
    # Whi*xhi + Whi*xlo + Wlo*xhi (residual ~2^-22) in one K=102 fp16 MM
    whi = wih1T.astype(np.float16)
    wlo = (wih1T - whi.astype(np.float32)).astype(np.float16)
    wih3T = np.concatenate([whi, whi, wlo], 0)  # (102, 512) fp16
    wkT = np.ascontiguousarray(Wk.T)  # (128, 128) fp32
    wkThi = wkT.astype(np.float16)
    wkTlo = (wkT - wkThi.astype(np.float32)).astype(np.float16)

    nc = _build_nc()
    shared = {
        "whhT": whhT.astype(np.float16),
        "wih3T": np.ascontiguousarray(wih3T),
        "wkThi": wkThi,
        "wkTlo": np.ascontiguousarray(wkTlo),
        "bkc": np.ascontiguousarray(bk[:, None]),
        "qT": qT.astype(np.float16),
    }
    in_maps = []
    for c in range(NCORES):
        xs = x[:, c * SH : (c + 1) * SH]  # (T, 64)
        xa1 = np.empty((34, T * SH), np.float32)
        xa1[0] = xs.reshape(-1)
        xa1[1:33] = np.repeat(prev_act.T, SH, axis=1).reshape(32, T * SH)
        xa1[33] = 1.0
        xahi = xa1.astype(np.float16)
        xalo = (xa1 - xahi.astype(np.float32)).astype(np.float16)
        xa3 = np.concatenate([xahi, xalo, xahi], 0)  # (102, T*SH) fp16
        # block-diagonal paired s columns: col t has s_t in rows [64j, 64j+64)
        # for j = t%2, zeros elsewhere
        xTp = np.zeros((2 * SH, T), np.float16)
        xTp[0:SH, 0::2] = xs.T[:, 0::2]
        xTp[SH : 2 * SH, 1::2] = xs.T[:, 1::2]
        in_maps.append({**shared, "xa3": xa3, "xTp": xTp})

    res = run_bass_kernel_spmd(nc, in_maps, list(range(NCORES)), trace=TRACE[0])
    LAST_RESULTS[0] = res
    total = np.zeros((T, HIDDEN_DIM), np.float32)
    for c in range(NCORES):
        raw = res.results[c]["outs"]  # (T//64, 128, 512)
        # col = (pair%32)*16 + hidden_block*2 + step_in_pair
        total += np.transpose(
            raw.reshape(T // 64, 128, 32, 8, 2), (0, 2, 4, 3, 1)
        ).reshape(T, HIDDEN_DIM)
    return np.tanh(total).astype(np.float32)


# revision 49
# speedup vs baseline: 1.0029x; 1.0029x over previous
"""AttentionNeuronLayer Trainium2 kernel.

Strategy: the obs_dim rows of the LSTM state evolve fully independently
through the whole recurrence (each obs scalar is paired with the full
action vector); only the final `out = w @ s` contracts over obs rows, and
the final tanh is applied after that sum.  So we shard obs_dim 512 -> 64
rows per core across 8 cores with zero collectives: each core runs the
full T=256 recurrence on its 64 rows in a transposed layout
(pos_em/gate/msg dims on partitions, obs rows on the free dim), computes
its slice of w = tanh(q @ k.T / sqrt(d)) and the partial contraction
w.T @ s, and the host sums the 8 partial (T, hidden) results and applies
the final tanh.

The per-step critical path is latency-bound: one serial chain
PE(whh) -> Act(sigma) -> elementwise c-update -> Act(tanh c) ->
h16-mul -> PE per step, each cross-engine hop costing ~100ns of
semaphore visibility, and the Act engine carries the attention tail's
tanh(w) on top of its two chain ops.  In steady state every Act gap is
filled and the step settles at ~1.43us = sigma(398) + w-chunk(412) +
tct(238) + w-chunk(385), Act-saturated.  Scheduling choices:
  * the x/a input half of the gates is accumulated into the step's PSUM
    bank ahead of time as one accumulation group (first xa matmul opens
    it, last whh matmul closes it), so after h16 lands only the four
    27ns W_hh matmuls remain on the chain;
  * one merged 256-wide sigmoid covers i,f,2g,o (g doubled so
    tanh(g) = 2*sigmoid(2g)-1 folds into the c-update);
  * the whole c-update (u = 2*sig2g-1, m = u*i, c = f*c + m) and
    h16 = sigma_o * tct run on the otherwise-idle GPSIMD engine, whose
    elementwise ops carry no SBUF-access bubble; sigma outputs, t1, cT,
    and tct are fp16;
  * the attention tail is pipelined behind the recurrence: fp16-h kT
    matmuls (hi/lo Wk weights), k-bias folded into the DVE PSUM->SBUF
    cast, w matmuls (the second hidden-half scheduling-pinned behind the
    next whh so they never head-of-line block the chain on the in-order
    PE), and tanh(w) split into 272/240-wide chunks pinned into the two
    Act gaps of each step (after sigma and after tct);
  * poT out-accumulation flushes in 128-col quarters through the DVE's
    idle window; input DMAs are chunked and spread across queues so the
    recurrence starts ~3.5us in; a dummy-matmul burst pre-warms the PE
    clock ramp during the DMA wait.
"""

import sys

sys.path.insert(0, "/opt/trn_rl_repo")

import numpy as np

import bass_rust
import concourse.bass as bass
import concourse.tile as tile
from concourse import mybir
from concourse.vector_clock import ScopedClock
from concourse.bass_utils import run_bass_kernel_spmd

OBS_DIM = 512
ACT_DIM = 32
HIDDEN_DIM = 1024
MSG_DIM = 128
POS_EM_DIM = 128
T = 256
NCORES = 8
SH = OBS_DIM // NCORES  # 64 obs rows per core

F32 = mybir.dt.float32
F16 = mybir.dt.float16
AF = mybir.ActivationFunctionType
ALU = mybir.AluOpType

# gate blocks laid out [i, f, g, o]; g row pre-scaled by 2
_PERM = [0, 1, 2, 3]

TRACE = [False]  # test.py flips this for the profiled run
LAST_RESULTS = [None]
INPUT_NAMES = ("whhT", "wih3T", "wkThi", "wkTlo", "bkc", "qT", "xa3", "xTp")


def _patched_drain_and_barrier(self, tick_clock, wait_clock):
    # This walrus build rejects instructions carrying more than one
    # sync-wait command; Tile's tail drain aggregates one wait per live
    # proc.  Re-emit the waits on individual single-wait NOPs instead.
    nc = self.nc
    carrier = nc.sync.nop(nofuse=True)
    wait_clock.add_sem_waits(carrier.ins, ScopedClock({None: tick_clock.global_clock}))
    si = carrier.ins.sync_info
    waits = list(si.on_wait) if si is not None and si.on_wait else []
    if si is not None:
        carrier.ins.sync_info = mybir.SyncInfo(
            on_wait=[], on_update=list(si.on_update or [])
        )
    for w in waits:
        n2 = nc.sync.nop(nofuse=True)
        n2.ins.sync_info = mybir.SyncInfo(on_wait=[w], on_update=[])
    nc.sync.drain()
    nc.all_engine_barrier()
    popped = nc._tile_sem_poison_stack.pop()
    assert popped is self._sem_poison
    nc.clear_and_free_semaphores(list(self.sems.allocated().values()))
    nc.all_engine_barrier()


tile.TileContext._drain_and_barrier = _patched_drain_and_barrier


def _split_multi_waits(module):
    """This walrus build accepts at most one sync-wait command per
    instruction.  Move excess waits onto same-engine NoOps inserted just
    before the instruction — the engine stream is serial, so gating an
    earlier NoOp on the same conditions is equivalent (DMA triggers are
    issued by their engine in program order, so this holds for DMACopy
    too)."""
    import copy as _copy

    counter = [0]
    new_module = _copy.replace(module, functions=[])
    for function in module.functions:
        new_function = _copy.replace(function, blocks=[])
        new_function.set_allocations_from_list(function.allocations)
        for block in function.blocks:
            new_insts = []
            for inst in block.instructions:
                si = inst.sync_info
                waits = list(si.on_wait) if si is not None and si.on_wait else []
                if len(waits) > 1:
                    for w in waits[:-1]:
                        counter[0] += 1
                        nop = mybir.InstNoOp(
                            engine=inst.engine, name=f"I-ws{counter[0]}"
                        )
                        nop.sync_info = mybir.SyncInfo(on_wait=[w], on_update=[])
                        new_insts.append(nop)
                    inst.sync_info = mybir.SyncInfo(
                        on_wait=[waits[-1]], on_update=list(si.on_update or [])
                    )
                new_insts.append(inst)
            new_function.blocks.append(_copy.replace(block, instructions=new_insts))
        new_module.functions.append(new_function)
    return new_module


_NC_CACHE = {}


def _build_nc(split=True):
    if split in _NC_CACHE:
        return _NC_CACHE[split]
    nc = bass.Bass()
    whhT = nc.declare_dram_parameter("whhT", [POS_EM_DIM, 512], F16, isOutput=False)
    wih3T = nc.declare_dram_parameter("wih3T", [102, 512], F16, isOutput=False)
    wkThi = nc.declare_dram_parameter("wkThi", [POS_EM_DIM, MSG_DIM], F16, isOutput=False)
    wkTlo = nc.declare_dram_parameter("wkTlo", [POS_EM_DIM, MSG_DIM], F16, isOutput=False)
    bkc = nc.declare_dram_parameter("bkc", [MSG_DIM, 1], F32, isOutput=False)
    qT = nc.declare_dram_parameter("qT", [MSG_DIM, HIDDEN_DIM], F16, isOutput=False)
    xa3 = nc.declare_dram_parameter("xa3", [102, T * SH], F16, isOutput=False)
    xTp = nc.declare_dram_parameter("xTp", [2 * SH, T], F16, isOutput=False)
    # transposed out accumulation: 64 steps (32 pairs x 8 hidden blocks x
    # N=2) fill one (128, 512) PSUM bank; host decodes the layout
    outs = nc.declare_dram_parameter("outs", [T // 64, 128, 512], F32, isOutput=True)

    inv_scale = 1.0 / float(np.sqrt(np.float32(MSG_DIM)))

    with tile.TileContext(nc) as tc:
        with (
            tc.tile_pool(name="const", bufs=1) as const,
            tc.tile_pool(name="state", bufs=1) as state,
            tc.tile_pool(name="hs", bufs=4) as hsp,
            tc.tile_pool(name="work", bufs=3) as work,
            tc.tile_pool(name="wap", bufs=2) as wap,
            tc.tile_pool(name="stg", bufs=2) as stg,
            tc.tile_pool(name="pg", bufs=2, space="PSUM") as pgp,
            tc.tile_pool(name="pk", bufs=1, space="PSUM") as pkp,
            tc.tile_pool(name="pw", bufs=2, space="PSUM") as pwp,
            tc.tile_pool(name="po", bufs=1, space="PSUM") as pop,
        ):
            whhT_sb = const.tile([POS_EM_DIM, 512], F16)
            wih3T_sb = const.tile([102, 512], F16)
            wkThi_sb = const.tile([POS_EM_DIM, MSG_DIM], F16)
            wkTlo_sb = const.tile([POS_EM_DIM, MSG_DIM], F16)
            bkc_sb = const.tile([MSG_DIM, 1], F32)
            qT_sb = const.tile([MSG_DIM, HIDDEN_DIM], F16)
            xa3_sb = const.tile([102, T * SH], F16)
            xTp_sb = const.tile([2 * SH, T], F16)
            # the recurrence can start once the weights and the first slice
            # of xa3 are resident: front-load those on the SP queue and
            # spread the rest (xa3 streams far ahead of consumption) across
            # the other engines' DMA queues so nothing serializes behind
            # the 3.2MB xa3 transfer
            XQ = T * SH // 4
            # all bulk DMAs ride the SP queue (SP is otherwise idle until
            # the first out-flush at ~100us); only qT uses the Act queue
            # (lands well before the first w matmuls).  Pool/DVE must stay
            # clear: DMA transfers occupy the issuing engine in the cost
            # model, and the chain needs them from ~7us on.
            nc.sync.dma_start(out=whhT_sb[:], in_=whhT[:])
            nc.sync.dma_start(out=wih3T_sb[:], in_=wih3T[:])
            nc.sync.dma_start(out=xa3_sb[:, 0 : XQ // 2], in_=xa3[:, 0 : XQ // 2])
            nc.scalar.dma_start(out=qT_sb[:], in_=qT[:])
            nc.sync.dma_start(out=xa3_sb[:, XQ // 2 : XQ], in_=xa3[:, XQ // 2 : XQ])
            nc.sync.dma_start(out=wkThi_sb[:], in_=wkThi[:])
            nc.sync.dma_start(out=wkTlo_sb[:], in_=wkTlo[:])
            nc.sync.dma_start(out=bkc_sb[:], in_=bkc[:])
            nc.sync.dma_start(out=xTp_sb[:], in_=xTp[:])
            nc.sync.dma_start(out=xa3_sb[:, XQ : 2 * XQ], in_=xa3[:, XQ : 2 * XQ])
            nc.sync.dma_start(
                out=xa3_sb[:, 2 * XQ : 3 * XQ], in_=xa3[:, 2 * XQ : 3 * XQ]
            )
            nc.sync.dma_start(
                out=xa3_sb[:, 3 * XQ : 4 * XQ], in_=xa3[:, 3 * XQ : 4 * XQ]
            )

            cT = state.tile([POS_EM_DIM, SH], F16)
            h0 = const.tile([POS_EM_DIM, SH], F16)
            nc.vector.memset(cT[:], 0.0)
            nc.vector.memset(h0[:], 0.0)
            # warm the sigmoid/tanh act table while the input DMAs stream
            scr = const.tile([128, 1], F32)
            nc.scalar.activation(scr[:], h0[:, 0:1], AF.Sigmoid)

            poT = pop.tile([128, 512], F32)
            nc.vector.memset(poT[:], 0.0)

            # PE p-state pre-warm: ~3us of dummy matmuls while the input
            # DMAs stream, so the real recurrence starts at full clock
            # (the tensor engine ramps 0.65 -> 2.4 GHz over 3us of use)
            pwm = pkp.tile([MSG_DIM, 2 * SH], F32, name="pwm", tag="pk")
            for _ in range(60):
                nc.tensor.matmul(pwm[0:64, 0:64], h0[:, 0:64], h0[:, 0:64], start=True, stop=True)

            h16bufs = {}
            kbufs = {}
            wabufs = {}
            pgbufs = {}
            pending_flush = []
            whh_last = [None]
            so_cur = [None]

            def emit_xa(t):
                # x/a input half of step t's gates: no h dependence, runs in
                # PE idle time ahead of the chain.  One PSUM accumulation
                # group covers the whole step (zero regions are
                # bank-granular, so only one group may be open per bank):
                # the first xa matmul opens it, the last whh matmul closes
                # it.  Full-bank tile so nothing else shares the region.
                pg = pgp.tile([128, 512], F32, name=f"pg{t}", tag="pg")[:, 0:256]
                pgbufs[t] = pg
                for b in range(4):
                    nc.tensor.matmul(
                        pg[:, 64 * b : 64 * b + 64],
                        wih3T_sb[:, 128 * b : 128 * b + 128],
                        xa3_sb[:, SH * t : SH * t + SH],
                        start=(b == 0),
                        stop=False,
                    )

            def emit_kT(p):
                # k = Wk @ h for the step pair, fp16 h with hi/lo weights
                pk = pkp.tile([MSG_DIM, 2 * SH], F32, tag="pk")
                nc.tensor.matmul(
                    pk[:], wkThi_sb[:], h16bufs[p][:], start=True, stop=False
                )
                nc.tensor.matmul(
                    pk[:], wkTlo_sb[:], h16bufs[p][:], start=False, stop=True
                )
                # PSUM->SBUF fp16 cast with the k-bias folded in; GPSIMD
                # cannot read PSUM, so this rides the DVE's idle window
                # between the c-chain and the next step
                kTp = work.tile([MSG_DIM, 2 * SH], F16, name=f"kTp{p}", tag="kTp")
                nc.vector.tensor_scalar_add(kTp[:], pk[:], bkc_sb[:])
                kbufs[p] = kTp

            def emit_w_mms(p):
                # rows 0-63 = even step of the pair, 64-127 = odd step.
                # The second hidden-half's matmuls get scheduling edges
                # behind the NEXT step's whh matmuls (pending_wdep) so the
                # 213ns w matmuls never head-of-line block the chain on
                # the in-order PE.
                kTp = kbufs.pop(p)
                pw = pwp.tile([128, HIDDEN_DIM], F32, name=f"pw{p}", tag="pw")
                for h2 in range(2):
                    for jj in range(2):
                        tp = None if jj == 0 else (0, 64)
                        mm = nc.tensor.matmul(
                            pw[64 * jj : 64 * jj + 64, 512 * h2 : 512 * h2 + 512],
                            kTp[:, SH * jj : SH * jj + SH],
                            qT_sb[:, 512 * h2 : 512 * h2 + 512],
                            start=True,
                            stop=True,
                            tile_position=tp,
                        )
                        bass_rust.add_dep_helper(
                            mm.ins,
                            whh_last[0].ins,
                            sync=False,
                            reason="w mm after whh",
                        )
                wa = wap.tile([128, HIDDEN_DIM], F16, name=f"wa{p}", tag="wa")
                wabufs[p] = pw, wa

            def emit_wa_chunk(p, lo, hi, after=None, final=False):
                # tanh over one slice of w, sized to ride an Act gap of the
                # chain (320 cols behind sigma, 192 behind tct).  The
                # scheduler orders by readiness, so pin it behind the chain
                # op with a no-sync edge (same engine, program order is
                # enough).
                pw, wa = wabufs[p]
                ch = nc.scalar.activation(
                    wa[:, lo:hi], pw[:, lo:hi], AF.Tanh, scale=inv_scale
                )
                if after is not None:
                    bass_rust.add_dep_helper(
                        ch.ins, after.ins, sync=False, reason="wa chunk placement"
                    )
                if final:
                    wabufs[p] = wa

            def emit_out(p):
                # transposed: out rows = hidden sub-dim (M=128), col pair =
                # the two steps; 32 pairs accumulate in one PSUM bank
                wa = wabufs.pop(p)
                base = 16 * (p % 32)
                for i in range(8):
                    nc.tensor.matmul(
                        poT[:, base + 2 * i : base + 2 * i + 2],
                        wa[:, 128 * i : 128 * i + 128],
                        xTp_sb[:, 2 * p : 2 * p + 2],
                        start=True,
                        stop=True,
                    )
                if p % 8 == 7:
                    pending_flush.append((p // 32, (p % 32) // 8, None))

            emit_xa(0)
            emit_xa(1)
            for t in range(T):
                j = t % 2  # position within the step pair
                p = t // 2
                if j == 0:
                    h16bufs[p] = hsp.tile(
                        [POS_EM_DIM, 2 * SH], F16, name=f"h16p{p}", tag="h16p"
                    )
                h_prev = (
                    h0[:]
                    if t == 0
                    else (
                        h16bufs[p - 1][:, SH:] if j == 0 else h16bufs[p][:, 0:SH]
                    )
                )
                # ---- gates: W_hh half, on the chain ----
                pg = pgbufs.pop(t)
                for b in range(4):
                    wm = nc.tensor.matmul(
                        pg[:, 64 * b : 64 * b + 64],
                        whhT_sb[:, 128 * b : 128 * b + 128],
                        h_prev,
                        start=False,
                        stop=(b == 3),
                    )
                whh_last[0] = wm
                # ---- merged sigmoid over i,f,2g,o (fp16 out) ----
                sig = work.tile([128, 256], F16, tag="sig")
                sg = nc.scalar.activation(sig[:], pg[:], AF.Sigmoid)
                # attention-tail tanh chunk rides the Act gap here
                if p >= 2:
                    emit_wa_chunk(p - 2, 512 * j, 512 * j + 272, after=sg)
                # ---- c ----  (tanh(g) = 2*sigmoid(2g) - 1)
                t1 = work.tile([128, SH], F16, tag="t1")
                nc.gpsimd.tensor_scalar(
                    t1[:], sig[:, 128:192], 2.0, -1.0, ALU.mult, ALU.add
                )
                nc.gpsimd.tensor_mul(t1[:], t1[:], sig[:, 0:64])
                nc.gpsimd.tensor_mul(cT[:], sig[:, 64:128], cT[:])
                ca = nc.gpsimd.tensor_add(cT[:], cT[:], t1[:])
                if pending_flush:
                    g, hh, _ = pending_flush.pop(0)
                    if hh == 0:
                        so_cur[0] = stg.tile([128, 512], F32, name="so", tag="so")
                    so = so_cur[0]
                    cp = nc.vector.tensor_copy(
                        so[:, 128 * hh : 128 * hh + 128],
                        poT[:, 128 * hh : 128 * hh + 128],
                    )
                    bass_rust.add_dep_helper(
                        cp.ins, ca.ins, sync=False, reason="flush placement"
                    )
                    if hh == 3:
                        nc.sync.dma_start(
                            out=outs[g, :, 384:512], in_=so[:, 384:512]
                        )
                    elif hh == 2:
                        nc.sync.dma_start(
                            out=outs[g, :, 0:384], in_=so[:, 0:384]
                        )
                tct = work.tile([128, SH], F16, tag="tct")
                tc = nc.scalar.activation(tct[:], cT[:], AF.Tanh)
                if p >= 2:
                    emit_wa_chunk(
                        p - 2,
                        512 * j + 272,
                        512 * j + 512,
                        after=tc,
                        final=(j == 1),
                    )
                # ---- h16 = sigma_o * tanh(c), on GPSIMD ----
                nc.gpsimd.tensor_mul(
                    h16bufs[p][:, SH * j : SH * j + SH], sig[:, 192:256], tct[:]
                )
                # ---- pipelined attention tail + next xa prefetch ----
                if j == 1:
                    if p >= 1:
                        emit_kT(p - 1)
                        emit_w_mms(p - 1)
                    if p == T // 2 - 1:
                        # last pair: split the k/w cascade per step-half so
                        # only the odd half trails the final h16
                        kTp_l = work.tile(
                            [MSG_DIM, 2 * SH], F16, name="kTpl", tag="kTp"
                        )
                        pw_l = pwp.tile(
                            [128, HIDDEN_DIM], F32, name="pwl", tag="pw"
                        )
                        wa_l = wap.tile(
                            [128, HIDDEN_DIM], F16, name="wal", tag="wa"
                        )
                        wabufs[p] = pw_l, wa_l
                        for half in range(2):
                            pkh = pkp.tile(
                                [MSG_DIM, 2 * SH], F32, name=f"pkl{half}", tag="pk"
                            )
                            sl = slice(SH * half, SH * half + SH)
                            nc.tensor.matmul(
                                pkh[:, sl],
                                wkThi_sb[:],
                                h16bufs[p][:, sl],
                                start=True,
                                stop=False,
                            )
                            nc.tensor.matmul(
                                pkh[:, sl],
                                wkTlo_sb[:],
                                h16bufs[p][:, sl],
                                start=False,
                                stop=True,
                            )
                            nc.vector.tensor_scalar_add(
                                kTp_l[:, sl], pkh[:, sl], bkc_sb[:]
                            )
                            tp = None if half == 0 else (0, 64)
                            for h2 in range(2):
                                nc.tensor.matmul(
                                    pw_l[
                                        64 * half : 64 * half + 64,
                                        512 * h2 : 512 * h2 + 512,
                                    ],
                                    kTp_l[:, sl],
                                    qT_sb[:, 512 * h2 : 512 * h2 + 512],
                                    start=True,
                                    stop=True,
                                    tile_position=tp,
                                )
                    if p >= 2:
                        emit_out(p - 2)
                    h16bufs.pop(p - 2, None)
                if t + 2 < T:
                    emit_xa(t + 2)
            last = T // 2 - 1  # 127
            emit_wa_chunk(last - 1, 0, 512)
            emit_wa_chunk(last - 1, 512, 1024, final=True)
            emit_out(last - 1)
            emit_wa_chunk(last, 0, 1024, final=True)
            emit_out(last)
            if pending_flush:
                g, hh, _ = pending_flush[0]
                if hh == 0:
                    so_cur[0] = stg.tile([128, 512], F32, name="so", tag="so")
                so = so_cur[0]
                nc.vector.tensor_copy(so[:, 128 * hh : 512], poT[:, 128 * hh : 512])
                if hh <= 2:
                    nc.sync.dma_start(out=outs[g, :, 0:384], in_=so[:, 0:384])
                nc.sync.dma_start(out=outs[g, :, 384:512], in_=so[:, 384:512])
    if split:
        nc.m = _split_multi_waits(nc.m)
    _NC_CACHE[split] = nc
    return nc


def kernel(
    obs,
    prev_act,
    in_shift,
    in_scale,
    pos_embedding,
    W_ih,
    b_ih,
    W_hh,
    b_hh,
    Wq,
    bq,
    Wk,
    bk,
):
    obs = np.asarray(obs, np.float32)
    prev_act = np.asarray(prev_act, np.float32)
    in_shift = np.asarray(in_shift, np.float32)
    in_scale = np.asarray(in_scale, np.float32)
    pos_embedding = np.asarray(pos_embedding, np.float32)
    W_ih = np.asarray(W_ih, np.float32)
    b_ih = np.asarray(b_ih, np.float32)
    W_hh = np.asarray(W_hh, np.float32)
    b_hh = np.asarray(b_hh, np.float32)
    Wq = np.asarray(Wq, np.float32)
    bq = np.asarray(bq, np.float32)
    Wk = np.asarray(Wk, np.float32)
    bk = np.asarray(bk, np.float32)

    x = (obs - in_shift) / (in_scale + 1e-8)  # (T, 512)
    q = pos_embedding @ Wq.T + bq  # (1024, 128)
    qT = np.ascontiguousarray(q.T)  # (128, 1024)

    def blocks(mat_rows):  # reorder gate blocks to [i, f, g, o]
        return np.concatenate([mat_rows[128 * p : 128 * p + 128] for p in _PERM], 0)

    W_ih_r = blocks(W_ih)  # (512, 33)
    W_hh_r = blocks(W_hh)  # (512, 128)
    b_r = blocks((b_ih + b_hh)[:, None])[:, 0]  # (512,)
    # g block (cols 256:384 after reorder) doubled: tanh(g) = 2*sig(2g)-1
    gs = np.ones((512, 1), np.float32)
    gs[256:384] = 2.0
    W_ih_r = W_ih_r * gs
    W_hh_r = W_hh_r * gs
    b_r = b_r * gs[:, 0]

    whhT = np.ascontiguousarray(W_hh_r.T)  # (128, 512)
    wih1T = np.concatenate(
        [W_ih_r[:, 0:1].T, np.ascontiguousarray(W_ih_r[:, 1:33].T), b_r[None, :]], 0
    )  # (34, 512)
    # fp16 hi/lo folded into K: [Whi; Whi; Wlo] x [xhi; xlo; xhi] gives
    # Whi*xhi + Whi*xlo + Wlo*xhi (residual ~2^-22) in one K=102 fp16 MM
    whi = wih1T.astype(np.float16)
    wlo = (wih1T - whi.astype(np.float32)).astype(np.float16)
    wih3T = np.concatenate([whi, whi, wlo], 0)  # (102, 512) fp16
    wkT = np.ascontiguousarray(Wk.T)  # (128, 128) fp32
    wkThi = wkT.astype(np.float16)
    wkTlo = (wkT - wkThi.astype(np.float32)).astype(np.float16)

    nc = _build_nc()
    shared = {
        "whhT": whhT.astype(np.float16),
        "wih3T": np.ascontiguousarray(wih3T),
        "wkThi": wkThi,
        "wkTlo": np.ascontiguousarray(wkTlo),
        "bkc": np.ascontiguousarray(bk[:, None]),
        "qT": qT.astype(np.float16),
    }
    in_maps = []
    for c in range(NCORES):
        xs = x[:, c * SH : (c + 1) * SH]  # (T, 64)
        xa1 = np.empty((34, T * SH), np.float32)
        xa1[0] = xs.reshape(-1)
        xa1[1:33] = np.repeat(prev_act.T, SH, axis=1).reshape(32, T * SH)
        xa1[33] = 1.0
        xahi = xa1.astype(np.float16)
        xalo = (xa1 - xahi.astype(np.float32)).astype(np.float16)
        xa3 = np.concatenate([xahi, xalo, xahi], 0)  # (102, T*SH) fp16
        # block-diagonal paired s columns: col t has s_t in rows [64j, 64j+64)
        # for j = t%2, zeros elsewhere
        xTp = np.zeros((2 * SH, T), np.float16)
        xTp[0:SH, 0::2] = xs.T[:, 0::2]
        xTp[SH : 2 * SH, 1::2] = xs.T[:, 1::2]
        in_maps.append({**shared, "xa3": xa3, "xTp": xTp})

    res = run_bass_kernel_spmd(nc, in_maps, list(range(NCORES)), trace=TRACE[0])
    LAST_RESULTS[0] = res
    total = np.zeros((T, HIDDEN_DIM), np.float32)
    for c in range(NCORES):
        raw = res.results[c]["outs"]  # (T//64, 128, 512)
        # col = (pair%32)*16 + hidden_block*2 + step_in_pair
        total += np.transpose(
            raw.reshape(T // 64, 128, 32, 8, 2), (0, 2, 4, 3, 1)
        ).reshape(T, HIDDEN_DIM)
    return np.tanh(total).astype(np.float32)


# revision 54
# speedup vs baseline: 1.0038x; 1.0009x over previous
"""AttentionNeuronLayer Trainium2 kernel.

Strategy: the obs_dim rows of the LSTM state evolve fully independently
through the whole recurrence (each obs scalar is paired with the full
action vector); only the final `out = w @ s` contracts over obs rows, and
the final tanh is applied after that sum.  So we shard obs_dim 512 -> 64
rows per core across 8 cores with zero collectives: each core runs the
full T=256 recurrence on its 64 rows in a transposed layout
(pos_em/gate/msg dims on partitions, obs rows on the free dim), computes
its slice of w = tanh(q @ k.T / sqrt(d)) and the partial contraction
w.T @ s, and the host sums the 8 partial (T, hidden) results and applies
the final tanh.

The per-step critical path is latency-bound: one serial chain
PE(whh) -> Act(sigma) -> elementwise c-update -> Act(tanh c) ->
h16-mul -> PE per step, each cross-engine hop costing ~100ns of
semaphore visibility, and the Act engine carries the attention tail's
tanh(w) on top of its two chain ops.  In steady state every Act gap is
filled and the step settles at ~1.43us = sigma(398) + w-chunk(412) +
tct(238) + w-chunk(385), Act-saturated.  Scheduling choices:
  * the x/a input half of the gates is accumulated into the step's PSUM
    bank ahead of time as one accumulation group (first xa matmul opens
    it, last whh matmul closes it), so after h16 lands only the four
    27ns W_hh matmuls remain on the chain;
  * one merged 256-wide sigmoid covers i,f,2g,o (g doubled so
    tanh(g) = 2*sigmoid(2g)-1 folds into the c-update);
  * the whole c-update (u = 2*sig2g-1, m = u*i, c = f*c + m) and
    h16 = sigma_o * tct run on the otherwise-idle GPSIMD engine, whose
    elementwise ops carry no SBUF-access bubble; sigma outputs, t1, cT,
    and tct are fp16;
  * the attention tail is pipelined behind the recurrence: fp16-h kT
    matmuls (hi/lo Wk weights), k-bias folded into the DVE PSUM->SBUF
    cast, w matmuls (the second hidden-half scheduling-pinned behind the
    next whh so they never head-of-line block the chain on the in-order
    PE), and tanh(w) split into 272/240-wide chunks pinned into the two
    Act gaps of each step (after sigma and after tct);
  * poT out-accumulation flushes in 128-col quarters through the DVE's
    idle window; input DMAs are chunked and spread across queues so the
    recurrence starts ~3.5us in; a dummy-matmul burst pre-warms the PE
    clock ramp during the DMA wait.
"""

import sys

sys.path.insert(0, "/opt/trn_rl_repo")

import numpy as np

import bass_rust
import concourse.bass as bass
import concourse.tile as tile
from concourse import mybir
from concourse.vector_clock import ScopedClock
from concourse.bass_utils import run_bass_kernel_spmd

OBS_DIM = 512
ACT_DIM = 32
HIDDEN_DIM = 1024
MSG_DIM = 128
POS_EM_DIM = 128
T = 256
NCORES = 8
SH = OBS_DIM // NCORES  # 64 obs rows per core

F32 = mybir.dt.float32
F16 = mybir.dt.float16
AF = mybir.ActivationFunctionType
ALU = mybir.AluOpType

# gate blocks laid out [i, f, g, o]; g row pre-scaled by 2
_PERM = [0, 1, 2, 3]

TRACE = [False]  # test.py flips this for the profiled run
LAST_RESULTS = [None]
INPUT_NAMES = ("whhT", "wih3T", "wkThi", "wkTlo", "bkc", "qT", "xa3", "xTp")


def _patched_drain_and_barrier(self, tick_clock, wait_clock):
    # This walrus build rejects instructions carrying more than one
    # sync-wait command; Tile's tail drain aggregates one wait per live
    # proc.  Re-emit the waits on individual single-wait NOPs instead.
    nc = self.nc
    carrier = nc.sync.nop(nofuse=True)
    wait_clock.add_sem_waits(carrier.ins, ScopedClock({None: tick_clock.global_clock}))
    si = carrier.ins.sync_info
    waits = list(si.on_wait) if si is not None and si.on_wait else []
    if si is not None:
        carrier.ins.sync_info = mybir.SyncInfo(
            on_wait=[], on_update=list(si.on_update or [])
        )
    for w in waits:
        n2 = nc.sync.nop(nofuse=True)
        n2.ins.sync_info = mybir.SyncInfo(on_wait=[w], on_update=[])
    nc.sync.drain()
    nc.all_engine_barrier()
    popped = nc._tile_sem_poison_stack.pop()
    assert popped is self._sem_poison
    nc.clear_and_free_semaphores(list(self.sems.allocated().values()))
    nc.all_engine_barrier()


tile.TileContext._drain_and_barrier = _patched_drain_and_barrier


def _split_multi_waits(module):
    """This walrus build accepts at most one sync-wait command per
    instruction.  Move excess waits onto same-engine NoOps inserted just
    before the instruction — the engine stream is serial, so gating an
    earlier NoOp on the same conditions is equivalent (DMA triggers are
    issued by their engine in program order, so this holds for DMACopy
    too)."""
    import copy as _copy

    counter = [0]
    new_module = _copy.replace(module, functions=[])
    for function in module.functions:
        new_function = _copy.replace(function, blocks=[])
        new_function.set_allocations_from_list(function.allocations)
        for block in function.blocks:
            new_insts = []
            for inst in block.instructions:
                si = inst.sync_info
                waits = list(si.on_wait) if si is not None and si.on_wait else []
                if len(waits) > 1:
                    for w in waits[:-1]:
                        counter[0] += 1
                        nop = mybir.InstNoOp(
                            engine=inst.engine, name=f"I-ws{counter[0]}"
                        )
                        nop.sync_info = mybir.SyncInfo(on_wait=[w], on_update=[])
                        new_insts.append(nop)
                    inst.sync_info = mybir.SyncInfo(
                        on_wait=[waits[-1]], on_update=list(si.on_update or [])
                    )
                new_insts.append(inst)
            new_function.blocks.append(_copy.replace(block, instructions=new_insts))
        new_module.functions.append(new_function)
    return new_module


_NC_CACHE = {}


def _build_nc(split=True):
    if split in _NC_CACHE:
        return _NC_CACHE[split]
    nc = bass.Bass()
    whhT = nc.declare_dram_parameter("whhT", [POS_EM_DIM, 512], F16, isOutput=False)
    wih3T = nc.declare_dram_parameter("wih3T", [102, 512], F16, isOutput=False)
    wkThi = nc.declare_dram_parameter("wkThi", [POS_EM_DIM, MSG_DIM], F16, isOutput=False)
    wkTlo = nc.declare_dram_parameter("wkTlo", [POS_EM_DIM, MSG_DIM], F16, isOutput=False)
    bkc = nc.declare_dram_parameter("bkc", [MSG_DIM, 1], F32, isOutput=False)
    qT = nc.declare_dram_parameter("qT", [MSG_DIM, HIDDEN_DIM], F16, isOutput=False)
    xa3 = nc.declare_dram_parameter("xa3", [102, T * SH], F16, isOutput=False)
    xTp = nc.declare_dram_parameter("xTp", [2 * SH, T], F16, isOutput=False)
    # transposed out accumulation: 64 steps (32 pairs x 8 hidden blocks x
    # N=2) fill one (128, 512) PSUM bank; host decodes the layout
    outs = nc.declare_dram_parameter("outs", [T // 64, 128, 512], F32, isOutput=True)

    inv_scale = 1.0 / float(np.sqrt(np.float32(MSG_DIM)))

    with tile.TileContext(nc) as tc:
        with (
            tc.tile_pool(name="const", bufs=1) as const,
            tc.tile_pool(name="state", bufs=1) as state,
            tc.tile_pool(name="hs", bufs=4) as hsp,
            tc.tile_pool(name="work", bufs=3) as work,
            tc.tile_pool(name="wap", bufs=2) as wap,
            tc.tile_pool(name="stg", bufs=2) as stg,
            tc.tile_pool(name="pg", bufs=2, space="PSUM") as pgp,
            tc.tile_pool(name="pk", bufs=1, space="PSUM") as pkp,
            tc.tile_pool(name="pw", bufs=2, space="PSUM") as pwp,
            tc.tile_pool(name="po", bufs=1, space="PSUM") as pop,
        ):
            whhT_sb = const.tile([POS_EM_DIM, 512], F16)
            wih3T_sb = const.tile([102, 512], F16)
            wkThi_sb = const.tile([POS_EM_DIM, MSG_DIM], F16)
            wkTlo_sb = const.tile([POS_EM_DIM, MSG_DIM], F16)
            bkc_sb = const.tile([MSG_DIM, 1], F32)
            qT_sb = const.tile([MSG_DIM, HIDDEN_DIM], F16)
            xa3_sb = const.tile([102, T * SH], F16)
            xTp_sb = const.tile([2 * SH, T], F16)
            # the recurrence can start once the weights and the first slice
            # of xa3 are resident: front-load those on the SP queue and
            # spread the rest (xa3 streams far ahead of consumption) across
            # the other engines' DMA queues so nothing serializes behind
            # the 3.2MB xa3 transfer
            XQ = T * SH // 4
            # all bulk DMAs ride the SP queue (SP is otherwise idle until
            # the first out-flush at ~100us); only qT uses the Act queue
            # (lands well before the first w matmuls).  Pool/DVE must stay
            # clear: DMA transfers occupy the issuing engine in the cost
            # model, and the chain needs them from ~7us on.
            nc.sync.dma_start(out=whhT_sb[:], in_=whhT[:])
            nc.sync.dma_start(out=wih3T_sb[:], in_=wih3T[:])
            nc.sync.dma_start(out=xa3_sb[:, 0 : XQ // 2], in_=xa3[:, 0 : XQ // 2])
            nc.scalar.dma_start(out=qT_sb[:], in_=qT[:])
            nc.sync.dma_start(out=xa3_sb[:, XQ // 2 : XQ], in_=xa3[:, XQ // 2 : XQ])
            nc.sync.dma_start(out=wkThi_sb[:], in_=wkThi[:])
            nc.sync.dma_start(out=wkTlo_sb[:], in_=wkTlo[:])
            nc.sync.dma_start(out=bkc_sb[:], in_=bkc[:])
            nc.sync.dma_start(out=xTp_sb[:], in_=xTp[:])
            nc.sync.dma_start(out=xa3_sb[:, XQ : 2 * XQ], in_=xa3[:, XQ : 2 * XQ])
            nc.sync.dma_start(
                out=xa3_sb[:, 2 * XQ : 3 * XQ], in_=xa3[:, 2 * XQ : 3 * XQ]
            )
            nc.sync.dma_start(
                out=xa3_sb[:, 3 * XQ : 4 * XQ], in_=xa3[:, 3 * XQ : 4 * XQ]
            )

            cT = state.tile([POS_EM_DIM, SH], F16)
            h0 = const.tile([POS_EM_DIM, SH], F16)
            nc.vector.memset(cT[:], 0.0)
            nc.vector.memset(h0[:], 0.0)
            # warm the sigmoid/tanh act table while the input DMAs stream
            scr = const.tile([128, 1], F32)
            nc.scalar.activation(scr[:], h0[:, 0:1], AF.Sigmoid)

            poT = pop.tile([128, 512], F32)
            nc.vector.memset(poT[:], 0.0)

            # PE p-state pre-warm: ~3us of dummy matmuls while the input
            # DMAs stream, so the real recurrence starts at full clock
            # (the tensor engine ramps 0.65 -> 2.4 GHz over 3us of use)
            pwm = pkp.tile([MSG_DIM, 2 * SH], F32, name="pwm", tag="pk")
            for _ in range(60):
                nc.tensor.matmul(pwm[0:64, 0:64], h0[:, 0:64], h0[:, 0:64], start=True, stop=True)

            h16bufs = {}
            kbufs = {}
            wabufs = {}
            pgbufs = {}
            pending_flush = []
            whh_last = [None]
            tct_last = [None]
            so_cur = [None]

            def emit_xa(t):
                # x/a input half of step t's gates: no h dependence, runs in
                # PE idle time ahead of the chain.  One PSUM accumulation
                # group covers the whole step (zero regions are
                # bank-granular, so only one group may be open per bank):
                # the first xa matmul opens it, the last whh matmul closes
                # it.  Full-bank tile so nothing else shares the region.
                pg = pgp.tile([128, 512], F32, name=f"pg{t}", tag="pg")[:, 0:256]
                pgbufs[t] = pg
                for b in range(4):
                    nc.tensor.matmul(
                        pg[:, 64 * b : 64 * b + 64],
                        wih3T_sb[:, 128 * b : 128 * b + 128],
                        xa3_sb[:, SH * t : SH * t + SH],
                        start=(b == 0),
                        stop=False,
                    )

            def emit_kT(p):
                # k = Wk @ h for the step pair, fp16 h with hi/lo weights
                pk = pkp.tile([MSG_DIM, 2 * SH], F32, tag="pk")
                nc.tensor.matmul(
                    pk[:], wkThi_sb[:], h16bufs[p][:], start=True, stop=False
                )
                nc.tensor.matmul(
                    pk[:], wkTlo_sb[:], h16bufs[p][:], start=False, stop=True
                )
                # PSUM->SBUF fp16 cast with the k-bias folded in; GPSIMD
                # cannot read PSUM, so this rides the DVE's idle window
                # between the c-chain and the next step
                kTp = work.tile([MSG_DIM, 2 * SH], F16, name=f"kTp{p}", tag="kTp")
                nc.vector.tensor_scalar_add(kTp[:], pk[:], bkc_sb[:])
                kbufs[p] = kTp

            def emit_w_mms(p):
                # rows 0-63 = even step of the pair, 64-127 = odd step.
                # The second hidden-half's matmuls get scheduling edges
                # behind the NEXT step's whh matmuls (pending_wdep) so the
                # 213ns w matmuls never head-of-line block the chain on
                # the in-order PE.
                kTp = kbufs.pop(p)
                pw = pwp.tile([128, HIDDEN_DIM], F32, name=f"pw{p}", tag="pw")
                for h2 in range(2):
                    for jj in range(2):
                        tp = None if jj == 0 else (0, 64)
                        mm = nc.tensor.matmul(
                            pw[64 * jj : 64 * jj + 64, 512 * h2 : 512 * h2 + 512],
                            kTp[:, SH * jj : SH * jj + SH],
                            qT_sb[:, 512 * h2 : 512 * h2 + 512],
                            start=True,
                            stop=True,
                            tile_position=tp,
                        )
                        bass_rust.add_dep_helper(
                            mm.ins,
                            whh_last[0].ins,
                            sync=False,
                            reason="w mm after whh",
                        )
                wa = wap.tile([128, HIDDEN_DIM], F16, name=f"wa{p}", tag="wa")
                wabufs[p] = pw, wa

            def emit_wa_chunk(p, lo, hi, after=None, final=False):
                # tanh over one slice of w, sized to ride an Act gap of the
                # chain (320 cols behind sigma, 192 behind tct).  The
                # scheduler orders by readiness, so pin it behind the chain
                # op with a no-sync edge (same engine, program order is
                # enough).
                pw, wa = wabufs[p]
                ch = nc.scalar.activation(
                    wa[:, lo:hi], pw[:, lo:hi], AF.Tanh, scale=inv_scale
                )
                if after is not None:
                    bass_rust.add_dep_helper(
                        ch.ins, after.ins, sync=False, reason="wa chunk placement"
                    )
                if final:
                    wabufs[p] = wa

            def emit_out(p):
                # transposed: out rows = hidden sub-dim (M=128), col pair =
                # the two steps; 32 pairs accumulate in one PSUM bank
                wa = wabufs.pop(p)
                base = 16 * (p % 32)
                for i in range(8):
                    nc.tensor.matmul(
                        poT[:, base + 2 * i : base + 2 * i + 2],
                        wa[:, 128 * i : 128 * i + 128],
                        xTp_sb[:, 2 * p : 2 * p + 2],
                        start=True,
                        stop=True,
                    )
                if p % 8 == 7:
                    pending_flush.append((p // 32, (p % 32) // 8, None))

            emit_xa(0)
            emit_xa(1)
            for t in range(T):
                j = t % 2  # position within the step pair
                p = t // 2
                if j == 0:
                    h16bufs[p] = hsp.tile(
                        [POS_EM_DIM, 2 * SH], F16, name=f"h16p{p}", tag="h16p"
                    )
                h_prev = (
                    h0[:]
                    if t == 0
                    else (
                        h16bufs[p - 1][:, SH:] if j == 0 else h16bufs[p][:, 0:SH]
                    )
                )
                # ---- gates: W_hh half, on the chain ----
                pg = pgbufs.pop(t)
                for b in range(4):
                    wm = nc.tensor.matmul(
                        pg[:, 64 * b : 64 * b + 64],
                        whhT_sb[:, 128 * b : 128 * b + 128],
                        h_prev,
                        start=False,
                        stop=(b == 3),
                    )
                whh_last[0] = wm
                # ---- merged sigmoid over i,f,2g,o (fp16 out) ----
                sig = work.tile([128, 256], F16, tag="sig")
                sg = nc.scalar.activation(sig[:], pg[:], AF.Sigmoid)
                # attention-tail tanh chunk rides the Act gap here
                if p >= 2:
                    emit_wa_chunk(p - 2, 512 * j, 512 * j + 272, after=sg)
                # ---- c ----  (tanh(g) = 2*sigmoid(2g) - 1)
                t1 = work.tile([128, SH], F16, tag="t1")
                nc.gpsimd.tensor_scalar(
                    t1[:], sig[:, 128:192], 2.0, -1.0, ALU.mult, ALU.add
                )
                nc.gpsimd.tensor_mul(t1[:], t1[:], sig[:, 0:64])
                nc.gpsimd.tensor_mul(cT[:], sig[:, 64:128], cT[:])
                ca = nc.gpsimd.tensor_add(cT[:], cT[:], t1[:])
                if pending_flush:
                    g, hh, _ = pending_flush.pop(0)
                    if hh == 0:
                        so_cur[0] = stg.tile([128, 512], F32, name="so", tag="so")
                    so = so_cur[0]
                    cp = nc.vector.tensor_copy(
                        so[:, 128 * hh : 128 * hh + 128],
                        poT[:, 128 * hh : 128 * hh + 128],
                    )
                    bass_rust.add_dep_helper(
                        cp.ins, ca.ins, sync=False, reason="flush placement"
                    )
                    if hh == 3:
                        nc.sync.dma_start(
                            out=outs[g, :, 384:512], in_=so[:, 384:512]
                        )
                    elif hh == 2:
                        nc.sync.dma_start(
                            out=outs[g, :, 0:384], in_=so[:, 0:384]
                        )
                tct = work.tile([128, SH], F16, tag="tct")
                tc = nc.scalar.activation(tct[:], cT[:], AF.Tanh)
                tct_last[0] = tc
                if p >= 2:
                    emit_wa_chunk(
                        p - 2,
                        512 * j + 272,
                        512 * j + 512,
                        after=tc,
                        final=(j == 1),
                    )
                # ---- h16 = sigma_o * tanh(c), on GPSIMD ----
                nc.gpsimd.tensor_mul(
                    h16bufs[p][:, SH * j : SH * j + SH], sig[:, 192:256], tct[:]
                )
                # ---- pipelined attention tail + next xa prefetch ----
                if j == 1:
                    if p >= 1:
                        emit_kT(p - 1)
                        emit_w_mms(p - 1)
                    if p == T // 2 - 1:
                        # last pair: split the k/w cascade per step-half so
                        # only the odd half trails the final h16
                        kTp_l = work.tile(
                            [MSG_DIM, 2 * SH], F16, name="kTpl", tag="kTp"
                        )
                        pw_l = pwp.tile(
                            [128, HIDDEN_DIM], F32, name="pwl", tag="pw"
                        )
                        wa_l = wap.tile(
                            [128, HIDDEN_DIM], F16, name="wal", tag="wa"
                        )
                        wabufs[p] = pw_l, wa_l
                        w_jj0 = []
                        for half in range(2):
                            pkh = pkp.tile(
                                [MSG_DIM, 2 * SH], F32, name=f"pkl{half}", tag="pk"
                            )
                            sl = slice(SH * half, SH * half + SH)
                            k1 = nc.tensor.matmul(
                                pkh[:, sl],
                                wkThi_sb[:],
                                h16bufs[p][:, sl],
                                start=True,
                                stop=False,
                            )
                            nc.tensor.matmul(
                                pkh[:, sl],
                                wkTlo_sb[:],
                                h16bufs[p][:, sl],
                                start=False,
                                stop=True,
                            )
                            if half == 1:
                                for mm in w_jj0:
                                    bass_rust.add_dep_helper(
                                        k1.ins,
                                        mm.ins,
                                        sync=False,
                                        reason="kT-b after w-jj0",
                                    )
                            nc.vector.tensor_scalar_add(
                                kTp_l[:, sl], pkh[:, sl], bkc_sb[:]
                            )
                            tp = None if half == 0 else (0, 64)
                            for h2 in range(2):
                                mm = nc.tensor.matmul(
                                    pw_l[
                                        64 * half : 64 * half + 64,
                                        512 * h2 : 512 * h2 + 512,
                                    ],
                                    kTp_l[:, sl],
                                    qT_sb[:, 512 * h2 : 512 * h2 + 512],
                                    start=True,
                                    stop=True,
                                    tile_position=tp,
                                )
                                if half == 0:
                                    w_jj0.append(mm)
                    if p >= 2:
                        emit_out(p - 2)
                    h16bufs.pop(p - 2, None)
                if t + 2 < T:
                    emit_xa(t + 2)
            last = T // 2 - 1  # 127
            emit_wa_chunk(last - 1, 0, 512, after=tct_last[0])
            emit_wa_chunk(last - 1, 512, 1024, final=True)
            emit_out(last - 1)
            emit_wa_chunk(last, 0, 1024, final=True)
            emit_out(last)
            if pending_flush:
                g, hh, _ = pending_flush[0]
                if hh == 0:
                    so_cur[0] = stg.tile([128, 512], F32, name="so", tag="so")
                so = so_cur[0]
                nc.vector.tensor_copy(so[:, 128 * hh : 512], poT[:, 128 * hh : 512])
                if hh <= 2:
                    nc.sync.dma_start(out=outs[g, :, 0:384], in_=so[:, 0:384])
                nc.sync.dma_start(out=outs[g, :, 384:512], in_=so[:, 384:512])
    if split:
        nc.m = _split_multi_waits(nc.m)
    _NC_CACHE[split] = nc
    return nc


def kernel(
    obs,
    prev_act,
    in_shift,
    in_scale,
    pos_embedding,
    W_ih,
    b_ih,
    W_hh,
    b_hh,
    Wq,
    bq,
    Wk,
    bk,
):
    obs = np.asarray(obs, np.float32)
    prev_act = np.asarray(prev_act, np.float32)
    in_shift = np.asarray(in_shift, np.float32)
    in_scale = np.asarray(in_scale, np.float32)
    pos_embedding = np.asarray(pos_embedding, np.float32)
    W_ih = np.asarray(W_ih, np.float32)
    b_ih = np.asarray(b_ih, np.float32)
    W_hh = np.asarray(W_hh, np.float32)
    b_hh = np.asarray(b_hh, np.float32)
    Wq = np.asarray(Wq, np.float32)
    bq = np.asarray(bq, np.float32)
    Wk = np.asarray(Wk, np.float32)
    bk = np.asarray(bk, np.float32)

    x = (obs - in_shift) / (in_scale + 1e-8)  # (T, 512)
    q = pos_embedding @ Wq.T + bq  # (1024, 128)
    qT = np.ascontiguousarray(q.T)  # (128, 1024)

    def blocks(mat_rows):  # reorder gate blocks to [i, f, g, o]
        return np.concatenate([mat_rows[128 * p : 128 * p + 128] for p in _PERM], 0)

    W_ih_r = blocks(W_ih)  # (512, 33)
    W_hh_r = blocks(W_hh)  # (512, 128)
    b_r = blocks((b_ih + b_hh)[:, None])[:, 0]  # (512,)
    # g block (cols 256:384 after reorder) doubled: tanh(g) = 2*sig(2g)-1
    gs = np.ones((512, 1), np.float32)
    gs[256:384] = 2.0
    W_ih_r = W_ih_r * gs
    W_hh_r = W_hh_r * gs
    b_r = b_r * gs[:, 0]

    whhT = np.ascontiguousarray(W_hh_r.T)  # (128, 512)
    wih1T = np.concatenate(
        [W_ih_r[:, 0:1].T, np.ascontiguousarray(W_ih_r[:, 1:33].T), b_r[None, :]], 0
    )  # (34, 512)
    # fp16 hi/lo folded into K: [Whi; Whi; Wlo] x [xhi; xlo; xhi] gives
    # Whi*xhi + Whi*xlo + Wlo*xhi (residual ~2^-22) in one K=102 fp16 MM
    whi = wih1T.astype(np.float16)
    wlo = (wih1T - whi.astype(np.float32)).astype(np.float16)
    wih3T = np.concatenate([whi, whi, wlo], 0)  # (102, 512) fp16
    wkT = np.ascontiguousarray(Wk.T)  # (128, 128) fp32
    wkThi = wkT.astype(np.float16)
    wkTlo = (wkT - wkThi.astype(np.float32)).astype(np.float16)

    nc = _build_nc()
    shared = {
        "whhT": whhT.astype(np.float16),
        "wih3T": np.ascontiguousarray(wih3T),
        "wkThi": wkThi,
        "wkTlo": np.ascontiguousarray(wkTlo),
        "bkc": np.ascontiguousarray(bk[:, None]),
        "qT": qT.astype(np.float16),
    }
    in_maps = []
    for c in range(NCORES):
        xs = x[:, c * SH : (c + 1) * SH]  # (T, 64)
        xa1 = np.empty((34, T * SH), np.float32)
        xa1[0] = xs.reshape(-1)
        xa1[1:33] = np.repeat(prev_act.T, SH, axis=1).reshape(32, T * SH)
        xa1[33] = 1.0
        xahi = xa1.astype(np.float16)
        xalo = (xa1 - xahi.astype(np.float32)).astype(np.float16)
        xa3 = np.concatenate([xahi, xalo, xahi], 0)  # (102, T*SH) fp16
        # block-diagonal paired s columns: col t has s_t in rows [64j, 64j+64)
        # for j = t%2, zeros elsewhere
        xTp = np.zeros((2 * SH, T), np.float16)
        xTp[0:SH, 0::2] = xs.T[:, 0::2]
        xTp[SH : 2 * SH, 1::2] = xs.T[:, 1::2]
        in_maps.append({**shared, "xa3": xa3, "xTp": xTp})

    res = run_bass_kernel_spmd(nc, in_maps, list(range(NCORES)), trace=TRACE[0])
    LAST_RESULTS[0] = res
    total = np.zeros((T, HIDDEN_DIM), np.float32)
    for c in range(NCORES):
        raw = res.results[c]["outs"]  # (T//64, 128, 512)
        # col = (pair%32)*16 + hidden_block*2 + step_in_pair
        total += np.transpose(
            raw.reshape(T // 64, 128, 32, 8, 2), (0, 2, 4, 3, 1)
        ).reshape(T, HIDDEN_DIM)
    return np.tanh(total).astype(np.float32)


# revision 55
# speedup vs baseline: 1.0043x; 1.0006x over previous
"""AttentionNeuronLayer Trainium2 kernel.

Strategy: the obs_dim rows of the LSTM state evolve fully independently
through the whole recurrence (each obs scalar is paired with the full
action vector); only the final `out = w @ s` contracts over obs rows, and
the final tanh is applied after that sum.  So we shard obs_dim 512 -> 64
rows per core across 8 cores with zero collectives: each core runs the
full T=256 recurrence on its 64 rows in a transposed layout
(pos_em/gate/msg dims on partitions, obs rows on the free dim), computes
its slice of w = tanh(q @ k.T / sqrt(d)) and the partial contraction
w.T @ s, and the host sums the 8 partial (T, hidden) results and applies
the final tanh.

The per-step critical path is latency-bound: one serial chain
PE(whh) -> Act(sigma) -> elementwise c-update -> Act(tanh c) ->
h16-mul -> PE per step, each cross-engine hop costing ~100ns of
semaphore visibility, and the Act engine carries the attention tail's
tanh(w) on top of its two chain ops.  In steady state every Act gap is
filled and the step settles at ~1.43us = sigma(398) + w-chunk(412) +
tct(238) + w-chunk(385), Act-saturated.  Scheduling choices:
  * the x/a input half of the gates is accumulated into the step's PSUM
    bank ahead of time as one accumulation group (first xa matmul opens
    it, last whh matmul closes it), so after h16 lands only the four
    27ns W_hh matmuls remain on the chain;
  * one merged 256-wide sigmoid covers i,f,2g,o (g doubled so
    tanh(g) = 2*sigmoid(2g)-1 folds into the c-update);
  * the whole c-update (u = 2*sig2g-1, m = u*i, c = f*c + m) and
    h16 = sigma_o * tct run on the otherwise-idle GPSIMD engine, whose
    elementwise ops carry no SBUF-access bubble; sigma outputs, t1, cT,
    and tct are fp16;
  * the attention tail is pipelined behind the recurrence: fp16-h kT
    matmuls (hi/lo Wk weights), k-bias folded into the DVE PSUM->SBUF
    cast, w matmuls (the second hidden-half scheduling-pinned behind the
    next whh so they never head-of-line block the chain on the in-order
    PE), and tanh(w) split into 272/240-wide chunks pinned into the two
    Act gaps of each step (after sigma and after tct);
  * poT out-accumulation flushes in 128-col quarters through the DVE's
    idle window; input DMAs are chunked and spread across queues so the
    recurrence starts ~3.5us in; a dummy-matmul burst pre-warms the PE
    clock ramp during the DMA wait.
"""

import sys

sys.path.insert(0, "/opt/trn_rl_repo")

import numpy as np

import bass_rust
import concourse.bass as bass
import concourse.tile as tile
from concourse import mybir
from concourse.vector_clock import ScopedClock
from concourse.bass_utils import run_bass_kernel_spmd

OBS_DIM = 512
ACT_DIM = 32
HIDDEN_DIM = 1024
MSG_DIM = 128
POS_EM_DIM = 128
T = 256
NCORES = 8
SH = OBS_DIM // NCORES  # 64 obs rows per core

F32 = mybir.dt.float32
F16 = mybir.dt.float16
AF = mybir.ActivationFunctionType
ALU = mybir.AluOpType

# gate blocks laid out [i, f, g, o]; g row pre-scaled by 2
_PERM = [0, 1, 2, 3]

TRACE = [False]  # test.py flips this for the profiled run
LAST_RESULTS = [None]
INPUT_NAMES = ("whhT", "wih3T", "wkThi", "wkTlo", "bkc", "qT", "xa3", "xTp")


def _patched_drain_and_barrier(self, tick_clock, wait_clock):
    # This walrus build rejects instructions carrying more than one
    # sync-wait command; Tile's tail drain aggregates one wait per live
    # proc.  Re-emit the waits on individual single-wait NOPs instead.
    nc = self.nc
    carrier = nc.sync.nop(nofuse=True)
    wait_clock.add_sem_waits(carrier.ins, ScopedClock({None: tick_clock.global_clock}))
    si = carrier.ins.sync_info
    waits = list(si.on_wait) if si is not None and si.on_wait else []
    if si is not None:
        carrier.ins.sync_info = mybir.SyncInfo(
            on_wait=[], on_update=list(si.on_update or [])
        )
    for w in waits:
        n2 = nc.sync.nop(nofuse=True)
        n2.ins.sync_info = mybir.SyncInfo(on_wait=[w], on_update=[])
    nc.sync.drain()
    nc.all_engine_barrier()
    popped = nc._tile_sem_poison_stack.pop()
    assert popped is self._sem_poison
    nc.clear_and_free_semaphores(list(self.sems.allocated().values()))
    nc.all_engine_barrier()


tile.TileContext._drain_and_barrier = _patched_drain_and_barrier


def _split_multi_waits(module):
    """This walrus build accepts at most one sync-wait command per
    instruction.  Move excess waits onto same-engine NoOps inserted just
    before the instruction — the engine stream is serial, so gating an
    earlier NoOp on the same conditions is equivalent (DMA triggers are
    issued by their engine in program order, so this holds for DMACopy
    too)."""
    import copy as _copy

    counter = [0]
    new_module = _copy.replace(module, functions=[])
    for function in module.functions:
        new_function = _copy.replace(function, blocks=[])
        new_function.set_allocations_from_list(function.allocations)
        for block in function.blocks:
            new_insts = []
            for inst in block.instructions:
                si = inst.sync_info
                waits = list(si.on_wait) if si is not None and si.on_wait else []
                if len(waits) > 1:
                    for w in waits[:-1]:
                        counter[0] += 1
                        nop = mybir.InstNoOp(
                            engine=inst.engine, name=f"I-ws{counter[0]}"
                        )
                        nop.sync_info = mybir.SyncInfo(on_wait=[w], on_update=[])
                        new_insts.append(nop)
                    inst.sync_info = mybir.SyncInfo(
                        on_wait=[waits[-1]], on_update=list(si.on_update or [])
                    )
                new_insts.append(inst)
            new_function.blocks.append(_copy.replace(block, instructions=new_insts))
        new_module.functions.append(new_function)
    return new_module


_NC_CACHE = {}


def _build_nc(split=True):
    if split in _NC_CACHE:
        return _NC_CACHE[split]
    nc = bass.Bass()
    whhT = nc.declare_dram_parameter("whhT", [POS_EM_DIM, 512], F16, isOutput=False)
    wih3T = nc.declare_dram_parameter("wih3T", [102, 512], F16, isOutput=False)
    wkThi = nc.declare_dram_parameter("wkThi", [POS_EM_DIM, MSG_DIM], F16, isOutput=False)
    wkTlo = nc.declare_dram_parameter("wkTlo", [POS_EM_DIM, MSG_DIM], F16, isOutput=False)
    bkc = nc.declare_dram_parameter("bkc", [MSG_DIM, 1], F32, isOutput=False)
    qT = nc.declare_dram_parameter("qT", [MSG_DIM, HIDDEN_DIM], F16, isOutput=False)
    xa3 = nc.declare_dram_parameter("xa3", [102, T * SH], F16, isOutput=False)
    xTp = nc.declare_dram_parameter("xTp", [2 * SH, T], F16, isOutput=False)
    # transposed out accumulation: 64 steps (32 pairs x 8 hidden blocks x
    # N=2) fill one (128, 512) PSUM bank; host decodes the layout
    outs = nc.declare_dram_parameter("outs", [T // 64, 128, 512], F32, isOutput=True)

    inv_scale = 1.0 / float(np.sqrt(np.float32(MSG_DIM)))

    with tile.TileContext(nc) as tc:
        with (
            tc.tile_pool(name="const", bufs=1) as const,
            tc.tile_pool(name="state", bufs=1) as state,
            tc.tile_pool(name="hs", bufs=4) as hsp,
            tc.tile_pool(name="work", bufs=3) as work,
            tc.tile_pool(name="wap", bufs=2) as wap,
            tc.tile_pool(name="stg", bufs=2) as stg,
            tc.tile_pool(name="pg", bufs=2, space="PSUM") as pgp,
            tc.tile_pool(name="pk", bufs=1, space="PSUM") as pkp,
            tc.tile_pool(name="pw", bufs=2, space="PSUM") as pwp,
            tc.tile_pool(name="po", bufs=1, space="PSUM") as pop,
        ):
            whhT_sb = const.tile([POS_EM_DIM, 512], F16)
            wih3T_sb = const.tile([102, 512], F16)
            wkThi_sb = const.tile([POS_EM_DIM, MSG_DIM], F16)
            wkTlo_sb = const.tile([POS_EM_DIM, MSG_DIM], F16)
            bkc_sb = const.tile([MSG_DIM, 1], F32)
            qT_sb = const.tile([MSG_DIM, HIDDEN_DIM], F16)
            xa3_sb = const.tile([102, T * SH], F16)
            xTp_sb = const.tile([2 * SH, T], F16)
            # the recurrence can start once the weights and the first slice
            # of xa3 are resident: front-load those on the SP queue and
            # spread the rest (xa3 streams far ahead of consumption) across
            # the other engines' DMA queues so nothing serializes behind
            # the 3.2MB xa3 transfer
            XQ = T * SH // 4
            # all bulk DMAs ride the SP queue (SP is otherwise idle until
            # the first out-flush at ~100us); only qT uses the Act queue
            # (lands well before the first w matmuls).  Pool/DVE must stay
            # clear: DMA transfers occupy the issuing engine in the cost
            # model, and the chain needs them from ~7us on.
            nc.sync.dma_start(out=whhT_sb[:], in_=whhT[:])
            nc.sync.dma_start(out=wih3T_sb[:], in_=wih3T[:])
            nc.sync.dma_start(out=xa3_sb[:, 0 : XQ // 2], in_=xa3[:, 0 : XQ // 2])
            nc.scalar.dma_start(out=qT_sb[:], in_=qT[:])
            nc.sync.dma_start(out=xa3_sb[:, XQ // 2 : XQ], in_=xa3[:, XQ // 2 : XQ])
            nc.sync.dma_start(out=wkThi_sb[:], in_=wkThi[:])
            nc.sync.dma_start(out=wkTlo_sb[:], in_=wkTlo[:])
            nc.sync.dma_start(out=bkc_sb[:], in_=bkc[:])
            nc.sync.dma_start(out=xTp_sb[:], in_=xTp[:])
            nc.sync.dma_start(out=xa3_sb[:, XQ : 2 * XQ], in_=xa3[:, XQ : 2 * XQ])
            nc.sync.dma_start(
                out=xa3_sb[:, 2 * XQ : 3 * XQ], in_=xa3[:, 2 * XQ : 3 * XQ]
            )
            nc.sync.dma_start(
                out=xa3_sb[:, 3 * XQ : 4 * XQ], in_=xa3[:, 3 * XQ : 4 * XQ]
            )

            cT = state.tile([POS_EM_DIM, SH], F16)
            h0 = const.tile([POS_EM_DIM, SH], F16)
            nc.vector.memset(cT[:], 0.0)
            nc.vector.memset(h0[:], 0.0)
            # warm the sigmoid/tanh act table while the input DMAs stream
            scr = const.tile([128, 1], F32)
            nc.scalar.activation(scr[:], h0[:, 0:1], AF.Sigmoid)

            poT = pop.tile([128, 512], F32)
            nc.vector.memset(poT[:], 0.0)

            # PE p-state pre-warm: ~3us of dummy matmuls while the input
            # DMAs stream, so the real recurrence starts at full clock
            # (the tensor engine ramps 0.65 -> 2.4 GHz over 3us of use)
            pwm = pkp.tile([MSG_DIM, 2 * SH], F32, name="pwm", tag="pk")
            for _ in range(60):
                nc.tensor.matmul(pwm[0:64, 0:64], h0[:, 0:64], h0[:, 0:64], start=True, stop=True)

            h16bufs = {}
            kbufs = {}
            wabufs = {}
            pgbufs = {}
            pending_flush = []
            whh_last = [None]
            tct_last = [None]
            so_cur = [None]

            def emit_xa(t):
                # x/a input half of step t's gates: no h dependence, runs in
                # PE idle time ahead of the chain.  One PSUM accumulation
                # group covers the whole step (zero regions are
                # bank-granular, so only one group may be open per bank):
                # the first xa matmul opens it, the last whh matmul closes
                # it.  Full-bank tile so nothing else shares the region.
                pg = pgp.tile([128, 512], F32, name=f"pg{t}", tag="pg")[:, 0:256]
                pgbufs[t] = pg
                for b in range(4):
                    nc.tensor.matmul(
                        pg[:, 64 * b : 64 * b + 64],
                        wih3T_sb[:, 128 * b : 128 * b + 128],
                        xa3_sb[:, SH * t : SH * t + SH],
                        start=(b == 0),
                        stop=False,
                    )

            def emit_kT(p):
                # k = Wk @ h for the step pair, fp16 h with hi/lo weights
                pk = pkp.tile([MSG_DIM, 2 * SH], F32, tag="pk")
                nc.tensor.matmul(
                    pk[:], wkThi_sb[:], h16bufs[p][:], start=True, stop=False
                )
                nc.tensor.matmul(
                    pk[:], wkTlo_sb[:], h16bufs[p][:], start=False, stop=True
                )
                # PSUM->SBUF fp16 cast with the k-bias folded in; GPSIMD
                # cannot read PSUM, so this rides the DVE's idle window
                # between the c-chain and the next step
                kTp = work.tile([MSG_DIM, 2 * SH], F16, name=f"kTp{p}", tag="kTp")
                nc.vector.tensor_scalar_add(kTp[:], pk[:], bkc_sb[:])
                kbufs[p] = kTp

            def emit_w_mms(p):
                # rows 0-63 = even step of the pair, 64-127 = odd step.
                # The second hidden-half's matmuls get scheduling edges
                # behind the NEXT step's whh matmuls (pending_wdep) so the
                # 213ns w matmuls never head-of-line block the chain on
                # the in-order PE.
                kTp = kbufs.pop(p)
                pw = pwp.tile([128, HIDDEN_DIM], F32, name=f"pw{p}", tag="pw")
                for h2 in range(2):
                    for jj in range(2):
                        tp = None if jj == 0 else (0, 64)
                        mm = nc.tensor.matmul(
                            pw[64 * jj : 64 * jj + 64, 512 * h2 : 512 * h2 + 512],
                            kTp[:, SH * jj : SH * jj + SH],
                            qT_sb[:, 512 * h2 : 512 * h2 + 512],
                            start=True,
                            stop=True,
                            tile_position=tp,
                        )
                        bass_rust.add_dep_helper(
                            mm.ins,
                            whh_last[0].ins,
                            sync=False,
                            reason="w mm after whh",
                        )
                wa = wap.tile([128, HIDDEN_DIM], F16, name=f"wa{p}", tag="wa")
                wabufs[p] = pw, wa

            def emit_wa_chunk(p, lo, hi, after=None, final=False):
                # tanh over one slice of w, sized to ride an Act gap of the
                # chain (320 cols behind sigma, 192 behind tct).  The
                # scheduler orders by readiness, so pin it behind the chain
                # op with a no-sync edge (same engine, program order is
                # enough).
                pw, wa = wabufs[p]
                ch = nc.scalar.activation(
                    wa[:, lo:hi], pw[:, lo:hi], AF.Tanh, scale=inv_scale
                )
                if after is not None:
                    bass_rust.add_dep_helper(
                        ch.ins, after.ins, sync=False, reason="wa chunk placement"
                    )
                if final:
                    wabufs[p] = wa

            def emit_out(p):
                # transposed: out rows = hidden sub-dim (M=128), col pair =
                # the two steps; 32 pairs accumulate in one PSUM bank
                wa = wabufs.pop(p)
                base = 16 * (p % 32)
                for i in range(8):
                    nc.tensor.matmul(
                        poT[:, base + 2 * i : base + 2 * i + 2],
                        wa[:, 128 * i : 128 * i + 128],
                        xTp_sb[:, 2 * p : 2 * p + 2],
                        start=True,
                        stop=True,
                    )
                if p % 8 == 7:
                    pending_flush.append((p // 32, (p % 32) // 8, None))

            emit_xa(0)
            emit_xa(1)
            for t in range(T):
                j = t % 2  # position within the step pair
                p = t // 2
                if j == 0:
                    h16bufs[p] = hsp.tile(
                        [POS_EM_DIM, 2 * SH], F16, name=f"h16p{p}", tag="h16p"
                    )
                h_prev = (
                    h0[:]
                    if t == 0
                    else (
                        h16bufs[p - 1][:, SH:] if j == 0 else h16bufs[p][:, 0:SH]
                    )
                )
                # ---- gates: W_hh half, on the chain ----
                pg = pgbufs.pop(t)
                for b in range(4):
                    wm = nc.tensor.matmul(
                        pg[:, 64 * b : 64 * b + 64],
                        whhT_sb[:, 128 * b : 128 * b + 128],
                        h_prev,
                        start=False,
                        stop=(b == 3),
                    )
                whh_last[0] = wm
                # ---- merged sigmoid over i,f,2g,o (fp16 out) ----
                sig = work.tile([128, 256], F16, tag="sig")
                sg = nc.scalar.activation(sig[:], pg[:], AF.Sigmoid)
                # attention-tail tanh chunk rides the Act gap here
                if p >= 2:
                    emit_wa_chunk(p - 2, 512 * j, 512 * j + 272, after=sg)
                # ---- c ----  (tanh(g) = 2*sigmoid(2g) - 1)
                t1 = work.tile([128, SH], F16, tag="t1")
                nc.gpsimd.tensor_scalar(
                    t1[:], sig[:, 128:192], 2.0, -1.0, ALU.mult, ALU.add
                )
                nc.gpsimd.tensor_mul(t1[:], t1[:], sig[:, 0:64])
                nc.gpsimd.tensor_mul(cT[:], sig[:, 64:128], cT[:])
                ca = nc.gpsimd.tensor_add(cT[:], cT[:], t1[:])
                if pending_flush:
                    g, hh, _ = pending_flush.pop(0)
                    if hh == 0:
                        so_cur[0] = stg.tile([128, 512], F32, name="so", tag="so")
                    so = so_cur[0]
                    cp = nc.vector.tensor_copy(
                        so[:, 128 * hh : 128 * hh + 128],
                        poT[:, 128 * hh : 128 * hh + 128],
                    )
                    bass_rust.add_dep_helper(
                        cp.ins, ca.ins, sync=False, reason="flush placement"
                    )
                    if hh == 3:
                        nc.sync.dma_start(
                            out=outs[g, :, 384:512], in_=so[:, 384:512]
                        )
                    elif hh == 2:
                        nc.sync.dma_start(
                            out=outs[g, :, 0:384], in_=so[:, 0:384]
                        )
                tct = work.tile([128, SH], F16, tag="tct")
                tc = nc.scalar.activation(tct[:], cT[:], AF.Tanh)
                tct_last[0] = tc
                if p >= 2:
                    emit_wa_chunk(
                        p - 2,
                        512 * j + 272,
                        512 * j + 512,
                        after=tc,
                        final=(j == 1),
                    )
                # ---- h16 = sigma_o * tanh(c), on GPSIMD ----
                nc.gpsimd.tensor_mul(
                    h16bufs[p][:, SH * j : SH * j + SH], sig[:, 192:256], tct[:]
                )
                # ---- pipelined attention tail + next xa prefetch ----
                if j == 1:
                    if p >= 1:
                        emit_kT(p - 1)
                        emit_w_mms(p - 1)
                    if p == T // 2 - 1:
                        # last pair: split the k/w cascade per step-half so
                        # only the odd half trails the final h16
                        kTp_l = work.tile(
                            [MSG_DIM, 2 * SH], F16, name="kTpl", tag="kTp"
                        )
                        pw_l = pwp.tile(
                            [128, HIDDEN_DIM], F32, name="pwl", tag="pw"
                        )
                        wa_l = wap.tile(
                            [128, HIDDEN_DIM], F16, name="wal", tag="wa"
                        )
                        wabufs[p] = pw_l, wa_l
                        w_jj0 = []
                        for half in range(2):
                            pkh = pkp.tile(
                                [MSG_DIM, 2 * SH], F32, name=f"pkl{half}", tag="pk"
                            )
                            sl = slice(SH * half, SH * half + SH)
                            nc.tensor.matmul(
                                pkh[:, sl],
                                wkThi_sb[:],
                                h16bufs[p][:, sl],
                                start=True,
                                stop=False,
                            )
                            k2 = nc.tensor.matmul(
                                pkh[:, sl],
                                wkTlo_sb[:],
                                h16bufs[p][:, sl],
                                start=False,
                                stop=True,
                            )
                            if half == 1:
                                for mm in w_jj0:
                                    bass_rust.add_dep_helper(
                                        mm.ins,
                                        k2.ins,
                                        sync=False,
                                        reason="w-jj0 after kT-b",
                                    )
                            nc.vector.tensor_scalar_add(
                                kTp_l[:, sl], pkh[:, sl], bkc_sb[:]
                            )
                            tp = None if half == 0 else (0, 64)
                            for h2 in range(2):
                                mm = nc.tensor.matmul(
                                    pw_l[
                                        64 * half : 64 * half + 64,
                                        512 * h2 : 512 * h2 + 512,
                                    ],
                                    kTp_l[:, sl],
                                    qT_sb[:, 512 * h2 : 512 * h2 + 512],
                                    start=True,
                                    stop=True,
                                    tile_position=tp,
                                )
                                if half == 0:
                                    w_jj0.append(mm)
                    if p >= 2:
                        emit_out(p - 2)
                    h16bufs.pop(p - 2, None)
                if t + 2 < T:
                    emit_xa(t + 2)
            last = T // 2 - 1  # 127
            emit_wa_chunk(last - 1, 0, 512, after=tct_last[0])
            emit_wa_chunk(last - 1, 512, 1024, final=True)
            emit_out(last - 1)
            emit_wa_chunk(last, 0, 1024, final=True)
            emit_out(last)
            if pending_flush:
                g, hh, _ = pending_flush[0]
                if hh == 0:
                    so_cur[0] = stg.tile([128, 512], F32, name="so", tag="so")
                so = so_cur[0]
                nc.vector.tensor_copy(so[:, 128 * hh : 512], poT[:, 128 * hh : 512])
                if hh <= 2:
                    nc.sync.dma_start(out=outs[g, :, 0:384], in_=so[:, 0:384])
                nc.sync.dma_start(out=outs[g, :, 384:512], in_=so[:, 384:512])
    if split:
        nc.m = _split_multi_waits(nc.m)
    _NC_CACHE[split] = nc
    return nc


def kernel(
    obs,
    prev_act,
    in_shift,
    in_scale,
    pos_embedding,
    W_ih,
    b_ih,
    W_hh,
    b_hh,
    Wq,
    bq,
    Wk,
    bk,
):
    obs = np.asarray(obs, np.float32)
    prev_act = np.asarray(prev_act, np.float32)
    in_shift = np.asarray(in_shift, np.float32)
    in_scale = np.asarray(in_scale, np.float32)
    pos_embedding = np.asarray(pos_embedding, np.float32)
    W_ih = np.asarray(W_ih, np.float32)
    b_ih = np.asarray(b_ih, np.float32)
    W_hh = np.asarray(W_hh, np.float32)
    b_hh = np.asarray(b_hh, np.float32)
    Wq = np.asarray(Wq, np.float32)
    bq = np.asarray(bq, np.float32)
    Wk = np.asarray(Wk, np.float32)
    bk = np.asarray(bk, np.float32)

    x = (obs - in_shift) / (in_scale + 1e-8)  # (T, 512)
    q = pos_embedding @ Wq.T + bq  # (1024, 128)
    qT = np.ascontiguousarray(q.T)  # (128, 1024)

    def blocks(mat_rows):  # reorder gate blocks to [i, f, g, o]
        return np.concatenate([mat_rows[128 * p : 128 * p + 128] for p in _PERM], 0)

    W_ih_r = blocks(W_ih)  # (512, 33)
    W_hh_r = blocks(W_hh)  # (512, 128)
    b_r = blocks((b_ih + b_hh)[:, None])[:, 0]  # (512,)
    # g block (cols 256:384 after reorder) doubled: tanh(g) = 2*sig(2g)-1
    gs = np.ones((512, 1), np.float32)
    gs[256:384] = 2.0
    W_ih_r = W_ih_r * gs
    W_hh_r = W_hh_r * gs
    b_r = b_r * gs[:, 0]

    whhT = np.ascontiguousarray(W_hh_r.T)  # (128, 512)
    wih1T = np.concatenate(
        [W_ih_r[:, 0:1].T, np.ascontiguousarray(W_ih_r[:, 1:33].T), b_r[None, :]], 0
    )  # (34, 512)
    # fp16 hi/lo folded into K: [Whi; Whi; Wlo] x [xhi; xlo; xhi] gives
    # Whi*xhi + Whi*xlo + Wlo*xhi (residual ~2^-22) in one K=102 fp16 MM
    whi = wih1T.astype(np.float16)
    wlo = (wih1T - whi.astype(np.float32)).astype(np.float16)
    wih3T = np.concatenate([whi, whi, wlo], 0)  # (102, 512) fp16
    wkT = np.ascontiguousarray(Wk.T)  # (128, 128) fp32
    wkThi = wkT.astype(np.float16)
    wkTlo = (wkT - wkThi.astype(np.float32)).astype(np.float16)

    nc = _build_nc()
    shared = {
        "whhT": whhT.astype(np.float16),
        "wih3T": np.ascontiguousarray(wih3T),
        "wkThi": wkThi,
        "wkTlo": np.ascontiguousarray(wkTlo),
        "bkc": np.ascontiguousarray(bk[:, None]),
        "qT": qT.astype(np.float16),
    }
    in_maps = []
    for c in range(NCORES):
        xs = x[:, c * SH : (c + 1) * SH]  # (T, 64)
        xa1 = np.empty((34, T * SH), np.float32)
        xa1[0] = xs.reshape(-1)
        xa1[1:33] = np.repeat(prev_act.T, SH, axis=1).reshape(32, T * SH)
        xa1[33] = 1.0
        xahi = xa1.astype(np.float16)
        xalo = (xa1 - xahi.astype(np.float32)).astype(np.float16)
        xa3 = np.concatenate([xahi, xalo, xahi], 0)  # (102, T*SH) fp16
        # block-diagonal paired s columns: col t has s_t in rows [64j, 64j+64)
        # for j = t%2, zeros elsewhere
        xTp = np.zeros((2 * SH, T), np.float16)
        xTp[0:SH, 0::2] = xs.T[:, 0::2]
        xTp[SH : 2 * SH, 1::2] = xs.T[:, 1::2]
        in_maps.append({**shared, "xa3": xa3, "xTp": xTp})

    res = run_bass_kernel_spmd(nc, in_maps, list(range(NCORES)), trace=TRACE[0])
    LAST_RESULTS[0] = res
    total = np.zeros((T, HIDDEN_DIM), np.float32)
    for c in range(NCORES):
        raw = res.results[c]["outs"]  # (T//64, 128, 512)
        # col = (pair%32)*16 + hidden_block*2 + step_in_pair
        total += np.transpose(
            raw.reshape(T // 64, 128, 32, 8, 2), (0, 2, 4, 3, 1)
        ).reshape(T, HIDDEN_DIM)
    return np.tanh(total).astype(np.float32)


# revision 59
# speedup vs baseline: 1.0049x; 1.0006x over previous
"""AttentionNeuronLayer Trainium2 kernel.

Strategy: the obs_dim rows of the LSTM state evolve fully independently
through the whole recurrence (each obs scalar is paired with the full
action vector); only the final `out = w @ s` contracts over obs rows, and
the final tanh is applied after that sum.  So we shard obs_dim 512 -> 64
rows per core across 8 cores with zero collectives: each core runs the
full T=256 recurrence on its 64 rows in a transposed layout
(pos_em/gate/msg dims on partitions, obs rows on the free dim), computes
its slice of w = tanh(q @ k.T / sqrt(d)) and the partial contraction
w.T @ s, and the host sums the 8 partial (T, hidden) results and applies
the final tanh.

The per-step critical path is latency-bound: one serial chain
PE(whh) -> Act(sigma) -> elementwise c-update -> Act(tanh c) ->
h16-mul -> PE per step, each cross-engine hop costing ~100ns of
semaphore visibility, and the Act engine carries the attention tail's
tanh(w) on top of its two chain ops.  In steady state every Act gap is
filled and the step settles at ~1.43us = sigma(398) + w-chunk(412) +
tct(238) + w-chunk(385), Act-saturated.  Scheduling choices:
  * the x/a input half of the gates is accumulated into the step's PSUM
    bank ahead of time as one accumulation group (first xa matmul opens
    it, last whh matmul closes it), so after h16 lands only the four
    27ns W_hh matmuls remain on the chain;
  * one merged 256-wide sigmoid covers i,f,2g,o (g doubled so
    tanh(g) = 2*sigmoid(2g)-1 folds into the c-update);
  * the whole c-update (u = 2*sig2g-1, m = u*i, c = f*c + m) and
    h16 = sigma_o * tct run on the otherwise-idle GPSIMD engine, whose
    elementwise ops carry no SBUF-access bubble; sigma outputs, t1, cT,
    and tct are fp16;
  * the attention tail is pipelined behind the recurrence: fp16-h kT
    matmuls (hi/lo Wk weights), k-bias folded into the DVE PSUM->SBUF
    cast, w matmuls (the second hidden-half scheduling-pinned behind the
    next whh so they never head-of-line block the chain on the in-order
    PE), and tanh(w) split into 272/240-wide chunks pinned into the two
    Act gaps of each step (after sigma and after tct);
  * poT out-accumulation flushes in 128-col quarters through the DVE's
    idle window; input DMAs are chunked and spread across queues so the
    recurrence starts ~3.5us in; a dummy-matmul burst pre-warms the PE
    clock ramp during the DMA wait.
"""

import sys

sys.path.insert(0, "/opt/trn_rl_repo")

import numpy as np

import bass_rust
import concourse.bass as bass
import concourse.tile as tile
from concourse import mybir
from concourse.vector_clock import ScopedClock
from concourse.bass_utils import run_bass_kernel_spmd

OBS_DIM = 512
ACT_DIM = 32
HIDDEN_DIM = 1024
MSG_DIM = 128
POS_EM_DIM = 128
T = 256
NCORES = 8
SH = OBS_DIM // NCORES  # 64 obs rows per core

F32 = mybir.dt.float32
F16 = mybir.dt.float16
AF = mybir.ActivationFunctionType
ALU = mybir.AluOpType

# gate blocks laid out [i, f, g, o]; g row pre-scaled by 2
_PERM = [0, 1, 2, 3]

TRACE = [False]  # test.py flips this for the profiled run
LAST_RESULTS = [None]
INPUT_NAMES = ("whhT", "wih3T", "wkThi", "wkTlo", "bkc", "qT", "xa3", "xTp")


def _patched_drain_and_barrier(self, tick_clock, wait_clock):
    # This walrus build rejects instructions carrying more than one
    # sync-wait command; Tile's tail drain aggregates one wait per live
    # proc.  Re-emit the waits on individual single-wait NOPs instead.
    nc = self.nc
    carrier = nc.sync.nop(nofuse=True)
    wait_clock.add_sem_waits(carrier.ins, ScopedClock({None: tick_clock.global_clock}))
    si = carrier.ins.sync_info
    waits = list(si.on_wait) if si is not None and si.on_wait else []
    if si is not None:
        carrier.ins.sync_info = mybir.SyncInfo(
            on_wait=[], on_update=list(si.on_update or [])
        )
    for w in waits:
        n2 = nc.sync.nop(nofuse=True)
        n2.ins.sync_info = mybir.SyncInfo(on_wait=[w], on_update=[])
    nc.sync.drain()
    nc.all_engine_barrier()
    popped = nc._tile_sem_poison_stack.pop()
    assert popped is self._sem_poison
    nc.clear_and_free_semaphores(list(self.sems.allocated().values()))
    nc.all_engine_barrier()


tile.TileContext._drain_and_barrier = _patched_drain_and_barrier


def _split_multi_waits(module):
    """This walrus build accepts at most one sync-wait command per
    instruction.  Move excess waits onto same-engine NoOps inserted just
    before the instruction — the engine stream is serial, so gating an
    earlier NoOp on the same conditions is equivalent (DMA triggers are
    issued by their engine in program order, so this holds for DMACopy
    too)."""
    import copy as _copy

    counter = [0]
    new_module = _copy.replace(module, functions=[])
    for function in module.functions:
        new_function = _copy.replace(function, blocks=[])
        new_function.set_allocations_from_list(function.allocations)
        for block in function.blocks:
            new_insts = []
            for inst in block.instructions:
                si = inst.sync_info
                waits = list(si.on_wait) if si is not None and si.on_wait else []
                if len(waits) > 1:
                    for w in waits[:-1]:
                        counter[0] += 1
                        nop = mybir.InstNoOp(
                            engine=inst.engine, name=f"I-ws{counter[0]}"
                        )
                        nop.sync_info = mybir.SyncInfo(on_wait=[w], on_update=[])
                        new_insts.append(nop)
                    inst.sync_info = mybir.SyncInfo(
                        on_wait=[waits[-1]], on_update=list(si.on_update or [])
                    )
                new_insts.append(inst)
            new_function.blocks.append(_copy.replace(block, instructions=new_insts))
        new_module.functions.append(new_function)
    return new_module


_NC_CACHE = {}


def _build_nc(split=True):
    if split in _NC_CACHE:
        return _NC_CACHE[split]
    nc = bass.Bass()
    whhT = nc.declare_dram_parameter("whhT", [POS_EM_DIM, 512], F16, isOutput=False)
    wih3T = nc.declare_dram_parameter("wih3T", [102, 512], F16, isOutput=False)
    wkThi = nc.declare_dram_parameter("wkThi", [POS_EM_DIM, MSG_DIM], F16, isOutput=False)
    wkTlo = nc.declare_dram_parameter("wkTlo", [POS_EM_DIM, MSG_DIM], F16, isOutput=False)
    bkc = nc.declare_dram_parameter("bkc", [MSG_DIM, 1], F32, isOutput=False)
    qT = nc.declare_dram_parameter("qT", [MSG_DIM, HIDDEN_DIM], F16, isOutput=False)
    xa3 = nc.declare_dram_parameter("xa3", [102, T * SH], F16, isOutput=False)
    xTp = nc.declare_dram_parameter("xTp", [2 * SH, T], F16, isOutput=False)
    # transposed out accumulation: 64 steps (32 pairs x 8 hidden blocks x
    # N=2) fill one (128, 512) PSUM bank; host decodes the layout
    outs = nc.declare_dram_parameter("outs", [T // 64, 128, 512], F32, isOutput=True)

    inv_scale = 1.0 / float(np.sqrt(np.float32(MSG_DIM)))

    with tile.TileContext(nc) as tc:
        with (
            tc.tile_pool(name="const", bufs=1) as const,
            tc.tile_pool(name="state", bufs=1) as state,
            tc.tile_pool(name="hs", bufs=4) as hsp,
            tc.tile_pool(name="work", bufs=3) as work,
            tc.tile_pool(name="wap", bufs=2) as wap,
            tc.tile_pool(name="stg", bufs=2) as stg,
            tc.tile_pool(name="pg", bufs=2, space="PSUM") as pgp,
            tc.tile_pool(name="pk", bufs=1, space="PSUM") as pkp,
            tc.tile_pool(name="pw", bufs=2, space="PSUM") as pwp,
            tc.tile_pool(name="po", bufs=1, space="PSUM") as pop,
        ):
            whhT_sb = const.tile([POS_EM_DIM, 512], F16)
            wih3T_sb = const.tile([102, 512], F16)
            wkThi_sb = const.tile([POS_EM_DIM, MSG_DIM], F16)
            wkTlo_sb = const.tile([POS_EM_DIM, MSG_DIM], F16)
            bkc_sb = const.tile([MSG_DIM, 1], F32)
            qT_sb = const.tile([MSG_DIM, HIDDEN_DIM], F16)
            xa3_sb = const.tile([102, T * SH], F16)
            xTp_sb = const.tile([2 * SH, T], F16)
            # the recurrence can start once the weights and the first slice
            # of xa3 are resident: front-load those on the SP queue and
            # spread the rest (xa3 streams far ahead of consumption) across
            # the other engines' DMA queues so nothing serializes behind
            # the 3.2MB xa3 transfer
            XQ = T * SH // 4
            # all bulk DMAs ride the SP queue (SP is otherwise idle until
            # the first out-flush at ~100us); only qT uses the Act queue
            # (lands well before the first w matmuls).  Pool/DVE must stay
            # clear: DMA transfers occupy the issuing engine in the cost
            # model, and the chain needs them from ~7us on.
            nc.sync.dma_start(out=whhT_sb[:], in_=whhT[:])
            nc.sync.dma_start(out=wih3T_sb[:], in_=wih3T[:])
            nc.sync.dma_start(out=xa3_sb[:, 0 : XQ // 2], in_=xa3[:, 0 : XQ // 2])
            nc.scalar.dma_start(out=qT_sb[:], in_=qT[:])
            nc.sync.dma_start(out=xa3_sb[:, XQ // 2 : XQ], in_=xa3[:, XQ // 2 : XQ])
            nc.sync.dma_start(out=wkThi_sb[:], in_=wkThi[:])
            nc.sync.dma_start(out=wkTlo_sb[:], in_=wkTlo[:])
            nc.sync.dma_start(out=bkc_sb[:], in_=bkc[:])
            nc.sync.dma_start(out=xTp_sb[:], in_=xTp[:])
            nc.sync.dma_start(out=xa3_sb[:, XQ : 2 * XQ], in_=xa3[:, XQ : 2 * XQ])
            nc.sync.dma_start(
                out=xa3_sb[:, 2 * XQ : 3 * XQ], in_=xa3[:, 2 * XQ : 3 * XQ]
            )
            nc.sync.dma_start(
                out=xa3_sb[:, 3 * XQ : 4 * XQ], in_=xa3[:, 3 * XQ : 4 * XQ]
            )

            cT = state.tile([POS_EM_DIM, SH], F16)
            h0 = const.tile([POS_EM_DIM, SH], F16)
            nc.vector.memset(cT[:], 0.0)
            nc.vector.memset(h0[:], 0.0)
            # warm the sigmoid/tanh act table while the input DMAs stream
            scr = const.tile([128, 1], F32)
            nc.scalar.activation(scr[:], h0[:, 0:1], AF.Sigmoid)

            poT = pop.tile([128, 512], F32)
            nc.vector.memset(poT[:], 0.0)

            # PE p-state pre-warm: ~3us of dummy matmuls while the input
            # DMAs stream, so the real recurrence starts at full clock
            # (the tensor engine ramps 0.65 -> 2.4 GHz over 3us of use)
            pwm = pkp.tile([MSG_DIM, 2 * SH], F32, name="pwm", tag="pk")
            for _ in range(46):
                nc.tensor.matmul(pwm[0:64, 0:64], h0[:, 0:64], h0[:, 0:64], start=True, stop=True)

            h16bufs = {}
            kbufs = {}
            wabufs = {}
            pgbufs = {}
            pending_flush = []
            whh_last = [None]
            tct_last = [None]
            so_cur = [None]

            def emit_xa(t):
                # x/a input half of step t's gates: no h dependence, runs in
                # PE idle time ahead of the chain.  One PSUM accumulation
                # group covers the whole step (zero regions are
                # bank-granular, so only one group may be open per bank):
                # the first xa matmul opens it, the last whh matmul closes
                # it.  Full-bank tile so nothing else shares the region.
                pg = pgp.tile([128, 512], F32, name=f"pg{t}", tag="pg")[:, 0:256]
                pgbufs[t] = pg
                for b in range(4):
                    nc.tensor.matmul(
                        pg[:, 64 * b : 64 * b + 64],
                        wih3T_sb[:, 128 * b : 128 * b + 128],
                        xa3_sb[:, SH * t : SH * t + SH],
                        start=(b == 0),
                        stop=False,
                    )

            def emit_kT(p):
                # k = Wk @ h for the step pair, fp16 h with hi/lo weights
                pk = pkp.tile([MSG_DIM, 2 * SH], F32, tag="pk")
                nc.tensor.matmul(
                    pk[:], wkThi_sb[:], h16bufs[p][:], start=True, stop=False
                )
                nc.tensor.matmul(
                    pk[:], wkTlo_sb[:], h16bufs[p][:], start=False, stop=True
                )
                # PSUM->SBUF fp16 cast with the k-bias folded in; GPSIMD
                # cannot read PSUM, so this rides the DVE's idle window
                # between the c-chain and the next step
                kTp = work.tile([MSG_DIM, 2 * SH], F16, name=f"kTp{p}", tag="kTp")
                nc.vector.tensor_scalar_add(kTp[:], pk[:], bkc_sb[:])
                kbufs[p] = kTp

            def emit_w_mms(p):
                # rows 0-63 = even step of the pair, 64-127 = odd step.
                # The second hidden-half's matmuls get scheduling edges
                # behind the NEXT step's whh matmuls (pending_wdep) so the
                # 213ns w matmuls never head-of-line block the chain on
                # the in-order PE.
                kTp = kbufs.pop(p)
                pw = pwp.tile([128, HIDDEN_DIM], F32, name=f"pw{p}", tag="pw")
                for h2 in range(2):
                    for jj in range(2):
                        tp = None if jj == 0 else (0, 64)
                        mm = nc.tensor.matmul(
                            pw[64 * jj : 64 * jj + 64, 512 * h2 : 512 * h2 + 512],
                            kTp[:, SH * jj : SH * jj + SH],
                            qT_sb[:, 512 * h2 : 512 * h2 + 512],
                            start=True,
                            stop=True,
                            tile_position=tp,
                        )
                        bass_rust.add_dep_helper(
                            mm.ins,
                            whh_last[0].ins,
                            sync=False,
                            reason="w mm after whh",
                        )
                wa = wap.tile([128, HIDDEN_DIM], F16, name=f"wa{p}", tag="wa")
                wabufs[p] = pw, wa

            def emit_wa_chunk(p, lo, hi, after=None, final=False):
                # tanh over one slice of w, sized to ride an Act gap of the
                # chain (320 cols behind sigma, 192 behind tct).  The
                # scheduler orders by readiness, so pin it behind the chain
                # op with a no-sync edge (same engine, program order is
                # enough).
                pw, wa = wabufs[p]
                ch = nc.scalar.activation(
                    wa[:, lo:hi], pw[:, lo:hi], AF.Tanh, scale=inv_scale
                )
                if after is not None:
                    bass_rust.add_dep_helper(
                        ch.ins, after.ins, sync=False, reason="wa chunk placement"
                    )
                if final:
                    wabufs[p] = wa

            def emit_out(p):
                # transposed: out rows = hidden sub-dim (M=128), col pair =
                # the two steps; 32 pairs accumulate in one PSUM bank
                wa = wabufs.pop(p)
                base = 16 * (p % 32)
                for i in range(8):
                    nc.tensor.matmul(
                        poT[:, base + 2 * i : base + 2 * i + 2],
                        wa[:, 128 * i : 128 * i + 128],
                        xTp_sb[:, 2 * p : 2 * p + 2],
                        start=True,
                        stop=True,
                    )
                if p % 8 == 7:
                    pending_flush.append((p // 32, (p % 32) // 8, None))

            emit_xa(0)
            emit_xa(1)
            for t in range(T):
                j = t % 2  # position within the step pair
                p = t // 2
                if j == 0:
                    h16bufs[p] = hsp.tile(
                        [POS_EM_DIM, 2 * SH], F16, name=f"h16p{p}", tag="h16p"
                    )
                h_prev = (
                    h0[:]
                    if t == 0
                    else (
                        h16bufs[p - 1][:, SH:] if j == 0 else h16bufs[p][:, 0:SH]
                    )
                )
                # ---- gates: W_hh half, on the chain ----
                pg = pgbufs.pop(t)
                for b in range(4):
                    wm = nc.tensor.matmul(
                        pg[:, 64 * b : 64 * b + 64],
                        whhT_sb[:, 128 * b : 128 * b + 128],
                        h_prev,
                        start=False,
                        stop=(b == 3),
                    )
                whh_last[0] = wm
                # ---- merged sigmoid over i,f,2g,o (fp16 out) ----
                sig = work.tile([128, 256], F16, tag="sig")
                sg = nc.scalar.activation(sig[:], pg[:], AF.Sigmoid)
                # attention-tail tanh chunk rides the Act gap here
                if p >= 2:
                    emit_wa_chunk(p - 2, 512 * j, 512 * j + 272, after=sg)
                # ---- c ----  (tanh(g) = 2*sigmoid(2g) - 1)
                t1 = work.tile([128, SH], F16, tag="t1")
                nc.gpsimd.tensor_scalar(
                    t1[:], sig[:, 128:192], 2.0, -1.0, ALU.mult, ALU.add
                )
                nc.gpsimd.tensor_mul(t1[:], t1[:], sig[:, 0:64])
                nc.gpsimd.tensor_mul(cT[:], sig[:, 64:128], cT[:])
                ca = nc.gpsimd.tensor_add(cT[:], cT[:], t1[:])
                if pending_flush:
                    g, hh, _ = pending_flush.pop(0)
                    if hh == 0:
                        so_cur[0] = stg.tile([128, 512], F32, name="so", tag="so")
                    so = so_cur[0]
                    cp = nc.vector.tensor_copy(
                        so[:, 128 * hh : 128 * hh + 128],
                        poT[:, 128 * hh : 128 * hh + 128],
                    )
                    bass_rust.add_dep_helper(
                        cp.ins, ca.ins, sync=False, reason="flush placement"
                    )
                    if hh == 3:
                        nc.sync.dma_start(
                            out=outs[g, :, 384:512], in_=so[:, 384:512]
                        )
                    elif hh == 2:
                        nc.sync.dma_start(
                            out=outs[g, :, 0:384], in_=so[:, 0:384]
                        )
                tct = work.tile([128, SH], F16, tag="tct")
                tc = nc.scalar.activation(tct[:], cT[:], AF.Tanh)
                tct_last[0] = tc
                if p >= 2:
                    emit_wa_chunk(
                        p - 2,
                        512 * j + 272,
                        512 * j + 512,
                        after=tc,
                        final=(j == 1),
                    )
                # ---- h16 = sigma_o * tanh(c), on GPSIMD ----
                nc.gpsimd.tensor_mul(
                    h16bufs[p][:, SH * j : SH * j + SH], sig[:, 192:256], tct[:]
                )
                # ---- pipelined attention tail + next xa prefetch ----
                if j == 1:
                    if p >= 1:
                        emit_kT(p - 1)
                        emit_w_mms(p - 1)
                    if p == T // 2 - 1:
                        # last pair: split the k/w cascade per step-half so
                        # only the odd half trails the final h16
                        kTp_l = work.tile(
                            [MSG_DIM, 2 * SH], F16, name="kTpl", tag="kTp"
                        )
                        pw_l = pwp.tile(
                            [128, HIDDEN_DIM], F32, name="pwl", tag="pw"
                        )
                        wa_l = wap.tile(
                            [128, HIDDEN_DIM], F16, name="wal", tag="wa"
                        )
                        wabufs[p] = pw_l, wa_l
                        w_jj0 = []
                        for half in range(2):
                            pkh = pkp.tile(
                                [MSG_DIM, 2 * SH], F32, name=f"pkl{half}", tag="pk"
                            )
                            sl = slice(SH * half, SH * half + SH)
                            nc.tensor.matmul(
                                pkh[:, sl],
                                wkThi_sb[:],
                                h16bufs[p][:, sl],
                                start=True,
                                stop=False,
                            )
                            k2 = nc.tensor.matmul(
                                pkh[:, sl],
                                wkTlo_sb[:],
                                h16bufs[p][:, sl],
                                start=False,
                                stop=True,
                            )
                            if half == 1:
                                for mm in w_jj0:
                                    bass_rust.add_dep_helper(
                                        mm.ins,
                                        k2.ins,
                                        sync=False,
                                        reason="w-jj0 after kT-b",
                                    )
                            nc.vector.tensor_scalar_add(
                                kTp_l[:, sl], pkh[:, sl], bkc_sb[:]
                            )
                            tp = None if half == 0 else (0, 64)
                            for h2 in range(2):
                                mm = nc.tensor.matmul(
                                    pw_l[
                                        64 * half : 64 * half + 64,
                                        512 * h2 : 512 * h2 + 512,
                                    ],
                                    kTp_l[:, sl],
                                    qT_sb[:, 512 * h2 : 512 * h2 + 512],
                                    start=True,
                                    stop=True,
                                    tile_position=tp,
                                )
                                if half == 0:
                                    w_jj0.append(mm)
                    if p >= 2:
                        emit_out(p - 2)
                    h16bufs.pop(p - 2, None)
                if t + 2 < T:
                    emit_xa(t + 2)
            last = T // 2 - 1  # 127
            emit_wa_chunk(last - 1, 0, 512, after=tct_last[0])
            emit_wa_chunk(last - 1, 512, 1024, final=True)
            emit_out(last - 1)
            emit_wa_chunk(last, 0, 1024, final=True)
            emit_out(last)
            if pending_flush:
                g, hh, _ = pending_flush[0]
                if hh == 0:
                    so_cur[0] = stg.tile([128, 512], F32, name="so", tag="so")
                so = so_cur[0]
                nc.vector.tensor_copy(so[:, 128 * hh : 512], poT[:, 128 * hh : 512])
                if hh <= 2:
                    nc.sync.dma_start(out=outs[g, :, 0:384], in_=so[:, 0:384])
                nc.sync.dma_start(out=outs[g, :, 384:512], in_=so[:, 384:512])
    if split:
        nc.m = _split_multi_waits(nc.m)
    _NC_CACHE[split] = nc
    return nc


def kernel(
    obs,
    prev_act,
    in_shift,
    in_scale,
    pos_embedding,
    W_ih,
    b_ih,
    W_hh,
    b_hh,
    Wq,
    bq,
    Wk,
    bk,
):
    obs = np.asarray(obs, np.float32)
    prev_act = np.asarray(prev_act, np.float32)
    in_shift = np.asarray(in_shift, np.float32)
    in_scale = np.asarray(in_scale, np.float32)
    pos_embedding = np.asarray(pos_embedding, np.float32)
    W_ih = np.asarray(W_ih, np.float32)
    b_ih = np.asarray(b_ih, np.float32)
    W_hh = np.asarray(W_hh, np.float32)
    b_hh = np.asarray(b_hh, np.float32)
    Wq = np.asarray(Wq, np.float32)
    bq = np.asarray(bq, np.float32)
    Wk = np.asarray(Wk, np.float32)
    bk = np.asarray(bk, np.float32)

    x = (obs - in_shift) / (in_scale + 1e-8)  # (T, 512)
    q = pos_embedding @ Wq.T + bq  # (1024, 128)
    qT = np.ascontiguousarray(q.T)  # (128, 1024)

    def blocks(mat_rows):  # reorder gate blocks to [i, f, g, o]
        return np.concatenate([mat_rows[128 * p : 128 * p + 128] for p in _PERM], 0)

    W_ih_r = blocks(W_ih)  # (512, 33)
    W_hh_r = blocks(W_hh)  # (512, 128)
    b_r = blocks((b_ih + b_hh)[:, None])[:, 0]  # (512,)
    # g block (cols 256:384 after reorder) doubled: tanh(g) = 2*sig(2g)-1
    gs = np.ones((512, 1), np.float32)
    gs[256:384] = 2.0
    W_ih_r = W_ih_r * gs
    W_hh_r = W_hh_r * gs
    b_r = b_r * gs[:, 0]

    whhT = np.ascontiguousarray(W_hh_r.T)  # (128, 512)
    wih1T = np.concatenate(
        [W_ih_r[:, 0:1].T, np.ascontiguousarray(W_ih_r[:, 1:33].T), b_r[None, :]], 0
    )  # (34, 512)
    # fp16 hi/lo folded into K: [Whi; Whi; Wlo] x [xhi; xlo; xhi] gives
    # Whi*xhi + Whi*xlo + Wlo*xhi (residual ~2^-22) in one K=102 fp16 MM
    whi = wih1T.astype(np.float16)
    wlo = (wih1T - whi.astype(np.float32)).astype(np.float16)
    wih3T = np.concatenate([whi, whi, wlo], 0)  # (102, 512) fp16
    wkT = np.ascontiguousarray(Wk.T)  # (128, 128) fp32
    wkThi = wkT.astype(np.float16)
    wkTlo = (wkT - wkThi.astype(np.float32)).astype(np.float16)

    nc = _build_nc()
    shared = {
        "whhT": whhT.astype(np.float16),
        "wih3T": np.ascontiguousarray(wih3T),
        "wkThi": wkThi,
        "wkTlo": np.ascontiguousarray(wkTlo),
        "bkc": np.ascontiguousarray(bk[:, None]),
        "qT": qT.astype(np.float16),
    }
    in_maps = []
    for c in range(NCORES):
        xs = x[:, c * SH : (c + 1) * SH]  # (T, 64)
        xa1 = np.empty((34, T * SH), np.float32)
        xa1[0] = xs.reshape(-1)
        xa1[1:33] = np.repeat(prev_act.T, SH, axis=1).reshape(32, T * SH)
        xa1[33] = 1.0
        xahi = xa1.astype(np.float16)
        xalo = (xa1 - xahi.astype(np.float32)).astype(np.float16)
        xa3 = np.concatenate([xahi, xalo, xahi], 0)  # (102, T*SH) fp16
        # block-diagonal paired s columns: col t has s_t in rows [64j, 64j+64)
        # for j = t%2, zeros elsewhere
        xTp = np.zeros((2 * SH, T), np.float16)
        xTp[0:SH, 0::2] = xs.T[:, 0::2]
        xTp[SH : 2 * SH, 1::2] = xs.T[:, 1::2]
        in_maps.append({**shared, "xa3": xa3, "xTp": xTp})

    res = run_bass_kernel_spmd(nc, in_maps, list(range(NCORES)), trace=TRACE[0])
    LAST_RESULTS[0] = res
    total = np.zeros((T, HIDDEN_DIM), np.float32)
    for c in range(NCORES):
        raw = res.results[c]["outs"]  # (T//64, 128, 512)
        # col = (pair%32)*16 + hidden_block*2 + step_in_pair
        total += np.transpose(
            raw.reshape(T // 64, 128, 32, 8, 2), (0, 2, 4, 3, 1)
        ).reshape(T, HIDDEN_DIM)
    return np.tanh(total).astype(np.float32)


# revision 63
# speedup vs baseline: 1.0056x; 1.0007x over previous
"""AttentionNeuronLayer Trainium2 kernel.

Strategy: the obs_dim rows of the LSTM state evolve fully independently
through the whole recurrence (each obs scalar is paired with the full
action vector); only the final `out = w @ s` contracts over obs rows, and
the final tanh is applied after that sum.  So we shard obs_dim 512 -> 64
rows per core across 8 cores with zero collectives: each core runs the
full T=256 recurrence on its 64 rows in a transposed layout
(pos_em/gate/msg dims on partitions, obs rows on the free dim), computes
its slice of w = tanh(q @ k.T / sqrt(d)) and the partial contraction
w.T @ s, and the host sums the 8 partial (T, hidden) results and applies
the final tanh.

The per-step critical path is latency-bound: one serial chain
PE(whh) -> Act(sigma) -> elementwise c-update -> Act(tanh c) ->
h16-mul -> PE per step, each cross-engine hop costing ~100ns of
semaphore visibility, and the Act engine carries the attention tail's
tanh(w) on top of its two chain ops.  In steady state every Act gap is
filled and the step settles at ~1.43us = sigma(398) + w-chunk(412) +
tct(238) + w-chunk(385), Act-saturated.  Scheduling choices:
  * the x/a input half of the gates is accumulated into the step's PSUM
    bank ahead of time as one accumulation group (first xa matmul opens
    it, last whh matmul closes it), so after h16 lands only the four
    27ns W_hh matmuls remain on the chain;
  * one merged 256-wide sigmoid covers i,f,2g,o (g doubled so
    tanh(g) = 2*sigmoid(2g)-1 folds into the c-update);
  * the whole c-update (u = 2*sig2g-1, m = u*i, c = f*c + m) and
    h16 = sigma_o * tct run on the otherwise-idle GPSIMD engine, whose
    elementwise ops carry no SBUF-access bubble; sigma outputs, t1, cT,
    and tct are fp16;
  * the attention tail is pipelined behind the recurrence: fp16-h kT
    matmuls (hi/lo Wk weights), k-bias folded into the DVE PSUM->SBUF
    cast, w matmuls (the second hidden-half scheduling-pinned behind the
    next whh so they never head-of-line block the chain on the in-order
    PE), and tanh(w) split into 274/238-wide chunks pinned into the two
    Act gaps of each step (after sigma and after tct);
  * poT out-accumulation flushes in 128-col quarters through the DVE's
    idle window; input DMAs are chunked and spread across queues so the
    recurrence starts ~3.5us in; a dummy-matmul burst pre-warms the PE
    clock ramp during the DMA wait.
"""

import sys

sys.path.insert(0, "/opt/trn_rl_repo")

import numpy as np

import bass_rust
import concourse.bass as bass
import concourse.tile as tile
from concourse import mybir
from concourse.vector_clock import ScopedClock
from concourse.bass_utils import run_bass_kernel_spmd

OBS_DIM = 512
ACT_DIM = 32
HIDDEN_DIM = 1024
MSG_DIM = 128
POS_EM_DIM = 128
T = 256
NCORES = 8
SH = OBS_DIM // NCORES  # 64 obs rows per core

F32 = mybir.dt.float32
F16 = mybir.dt.float16
AF = mybir.ActivationFunctionType
ALU = mybir.AluOpType

# gate blocks laid out [i, f, g, o]; g row pre-scaled by 2
_PERM = [0, 1, 2, 3]

TRACE = [False]  # test.py flips this for the profiled run
LAST_RESULTS = [None]
INPUT_NAMES = ("whhT", "wih3T", "wkThi", "wkTlo", "bkc", "qT", "xa3", "xTp")


def _patched_drain_and_barrier(self, tick_clock, wait_clock):
    # This walrus build rejects instructions carrying more than one
    # sync-wait command; Tile's tail drain aggregates one wait per live
    # proc.  Re-emit the waits on individual single-wait NOPs instead.
    nc = self.nc
    carrier = nc.sync.nop(nofuse=True)
    wait_clock.add_sem_waits(carrier.ins, ScopedClock({None: tick_clock.global_clock}))
    si = carrier.ins.sync_info
    waits = list(si.on_wait) if si is not None and si.on_wait else []
    if si is not None:
        carrier.ins.sync_info = mybir.SyncInfo(
            on_wait=[], on_update=list(si.on_update or [])
        )
    for w in waits:
        n2 = nc.sync.nop(nofuse=True)
        n2.ins.sync_info = mybir.SyncInfo(on_wait=[w], on_update=[])
    nc.sync.drain()
    nc.all_engine_barrier()
    popped = nc._tile_sem_poison_stack.pop()
    assert popped is self._sem_poison
    nc.clear_and_free_semaphores(list(self.sems.allocated().values()))
    nc.all_engine_barrier()


tile.TileContext._drain_and_barrier = _patched_drain_and_barrier


def _split_multi_waits(module):
    """This walrus build accepts at most one sync-wait command per
    instruction.  Move excess waits onto same-engine NoOps inserted just
    before the instruction — the engine stream is serial, so gating an
    earlier NoOp on the same conditions is equivalent (DMA triggers are
    issued by their engine in program order, so this holds for DMACopy
    too)."""
    import copy as _copy

    counter = [0]
    new_module = _copy.replace(module, functions=[])
    for function in module.functions:
        new_function = _copy.replace(function, blocks=[])
        new_function.set_allocations_from_list(function.allocations)
        for block in function.blocks:
            new_insts = []
            for inst in block.instructions:
                si = inst.sync_info
                waits = list(si.on_wait) if si is not None and si.on_wait else []
                if len(waits) > 1:
                    for w in waits[:-1]:
                        counter[0] += 1
                        nop = mybir.InstNoOp(
                            engine=inst.engine, name=f"I-ws{counter[0]}"
                        )
                        nop.sync_info = mybir.SyncInfo(on_wait=[w], on_update=[])
                        new_insts.append(nop)
                    inst.sync_info = mybir.SyncInfo(
                        on_wait=[waits[-1]], on_update=list(si.on_update or [])
                    )
                new_insts.append(inst)
            new_function.blocks.append(_copy.replace(block, instructions=new_insts))
        new_module.functions.append(new_function)
    return new_module


_NC_CACHE = {}


def _build_nc(split=True):
    if split in _NC_CACHE:
        return _NC_CACHE[split]
    nc = bass.Bass()
    whhT = nc.declare_dram_parameter("whhT", [POS_EM_DIM, 512], F16, isOutput=False)
    wih3T = nc.declare_dram_parameter("wih3T", [102, 512], F16, isOutput=False)
    wkThi = nc.declare_dram_parameter("wkThi", [POS_EM_DIM, MSG_DIM], F16, isOutput=False)
    wkTlo = nc.declare_dram_parameter("wkTlo", [POS_EM_DIM, MSG_DIM], F16, isOutput=False)
    bkc = nc.declare_dram_parameter("bkc", [MSG_DIM, 1], F32, isOutput=False)
    qT = nc.declare_dram_parameter("qT", [MSG_DIM, HIDDEN_DIM], F16, isOutput=False)
    xa3 = nc.declare_dram_parameter("xa3", [102, T * SH], F16, isOutput=False)
    xTp = nc.declare_dram_parameter("xTp", [2 * SH, T], F16, isOutput=False)
    # transposed out accumulation: 64 steps (32 pairs x 8 hidden blocks x
    # N=2) fill one (128, 512) PSUM bank; host decodes the layout
    outs = nc.declare_dram_parameter("outs", [T // 64, 128, 512], F32, isOutput=True)

    inv_scale = 1.0 / float(np.sqrt(np.float32(MSG_DIM)))

    with tile.TileContext(nc) as tc:
        with (
            tc.tile_pool(name="const", bufs=1) as const,
            tc.tile_pool(name="state", bufs=1) as state,
            tc.tile_pool(name="hs", bufs=4) as hsp,
            tc.tile_pool(name="work", bufs=3) as work,
            tc.tile_pool(name="wap", bufs=2) as wap,
            tc.tile_pool(name="stg", bufs=2) as stg,
            tc.tile_pool(name="pg", bufs=2, space="PSUM") as pgp,
            tc.tile_pool(name="pk", bufs=1, space="PSUM") as pkp,
            tc.tile_pool(name="pw", bufs=2, space="PSUM") as pwp,
            tc.tile_pool(name="po", bufs=1, space="PSUM") as pop,
        ):
            whhT_sb = const.tile([POS_EM_DIM, 512], F16)
            wih3T_sb = const.tile([102, 512], F16)
            wkThi_sb = const.tile([POS_EM_DIM, MSG_DIM], F16)
            wkTlo_sb = const.tile([POS_EM_DIM, MSG_DIM], F16)
            bkc_sb = const.tile([MSG_DIM, 1], F32)
            qT_sb = const.tile([MSG_DIM, HIDDEN_DIM], F16)
            xa3_sb = const.tile([102, T * SH], F16)
            xTp_sb = const.tile([2 * SH, T], F16)
            # the recurrence can start once the weights and the first slice
            # of xa3 are resident: front-load those on the SP queue and
            # spread the rest (xa3 streams far ahead of consumption) across
            # the other engines' DMA queues so nothing serializes behind
            # the 3.2MB xa3 transfer
            XQ = T * SH // 4
            # all bulk DMAs ride the SP queue (SP is otherwise idle until
            # the first out-flush at ~100us); only qT uses the Act queue
            # (lands well before the first w matmuls).  Pool/DVE must stay
            # clear: DMA transfers occupy the issuing engine in the cost
            # model, and the chain needs them from ~7us on.
            nc.sync.dma_start(out=whhT_sb[:], in_=whhT[:])
            nc.sync.dma_start(out=wih3T_sb[:], in_=wih3T[:])
            nc.sync.dma_start(out=xa3_sb[:, 0 : XQ // 2], in_=xa3[:, 0 : XQ // 2])
            nc.scalar.dma_start(out=qT_sb[:], in_=qT[:])
            nc.sync.dma_start(out=xa3_sb[:, XQ // 2 : XQ], in_=xa3[:, XQ // 2 : XQ])
            nc.sync.dma_start(out=wkThi_sb[:], in_=wkThi[:])
            nc.sync.dma_start(out=wkTlo_sb[:], in_=wkTlo[:])
            nc.sync.dma_start(out=bkc_sb[:], in_=bkc[:])
            nc.sync.dma_start(out=xTp_sb[:], in_=xTp[:])
            nc.sync.dma_start(out=xa3_sb[:, XQ : 2 * XQ], in_=xa3[:, XQ : 2 * XQ])
            nc.sync.dma_start(
                out=xa3_sb[:, 2 * XQ : 3 * XQ], in_=xa3[:, 2 * XQ : 3 * XQ]
            )
            nc.sync.dma_start(
                out=xa3_sb[:, 3 * XQ : 4 * XQ], in_=xa3[:, 3 * XQ : 4 * XQ]
            )

            cT = state.tile([POS_EM_DIM, SH], F16)
            h0 = const.tile([POS_EM_DIM, SH], F16)
            nc.vector.memset(cT[:], 0.0)
            nc.vector.memset(h0[:], 0.0)
            # warm the sigmoid/tanh act table while the input DMAs stream
            scr = const.tile([128, 1], F32)
            nc.scalar.activation(scr[:], h0[:, 0:1], AF.Sigmoid)

            poT = pop.tile([128, 512], F32)
            nc.vector.memset(poT[:], 0.0)

            # PE p-state pre-warm: ~3us of dummy matmuls while the input
            # DMAs stream, so the real recurrence starts at full clock
            # (the tensor engine ramps 0.65 -> 2.4 GHz over 3us of use)
            pwm = pkp.tile([MSG_DIM, 2 * SH], F32, name="pwm", tag="pk")
            for _ in range(46):
                nc.tensor.matmul(pwm[0:64, 0:64], h0[:, 0:64], h0[:, 0:64], start=True, stop=True)

            h16bufs = {}
            kbufs = {}
            wabufs = {}
            pgbufs = {}
            pending_flush = []
            whh_last = [None]
            tct_last = [None]
            so_cur = [None]

            def emit_xa(t):
                # x/a input half of step t's gates: no h dependence, runs in
                # PE idle time ahead of the chain.  One PSUM accumulation
                # group covers the whole step (zero regions are
                # bank-granular, so only one group may be open per bank):
                # the first xa matmul opens it, the last whh matmul closes
                # it.  Full-bank tile so nothing else shares the region.
                pg = pgp.tile([128, 512], F32, name=f"pg{t}", tag="pg")[:, 0:256]
                pgbufs[t] = pg
                for b in range(4):
                    nc.tensor.matmul(
                        pg[:, 64 * b : 64 * b + 64],
                        wih3T_sb[:, 128 * b : 128 * b + 128],
                        xa3_sb[:, SH * t : SH * t + SH],
                        start=(b == 0),
                        stop=False,
                    )

            def emit_kT(p):
                # k = Wk @ h for the step pair, fp16 h with hi/lo weights
                pk = pkp.tile([MSG_DIM, 2 * SH], F32, tag="pk")
                nc.tensor.matmul(
                    pk[:], wkThi_sb[:], h16bufs[p][:], start=True, stop=False
                )
                nc.tensor.matmul(
                    pk[:], wkTlo_sb[:], h16bufs[p][:], start=False, stop=True
                )
                # PSUM->SBUF fp16 cast with the k-bias folded in; GPSIMD
                # cannot read PSUM, so this rides the DVE's idle window
                # between the c-chain and the next step
                kTp = work.tile([MSG_DIM, 2 * SH], F16, name=f"kTp{p}", tag="kTp")
                nc.vector.tensor_scalar_add(kTp[:], pk[:], bkc_sb[:])
                kbufs[p] = kTp

            def emit_w_mms(p):
                # rows 0-63 = even step of the pair, 64-127 = odd step.
                # The second hidden-half's matmuls get scheduling edges
                # behind the NEXT step's whh matmuls (pending_wdep) so the
                # 213ns w matmuls never head-of-line block the chain on
                # the in-order PE.
                kTp = kbufs.pop(p)
                pw = pwp.tile([128, HIDDEN_DIM], F32, name=f"pw{p}", tag="pw")
                for h2 in range(2):
                    for jj in range(2):
                        tp = None if jj == 0 else (0, 64)
                        mm = nc.tensor.matmul(
                            pw[64 * jj : 64 * jj + 64, 512 * h2 : 512 * h2 + 512],
                            kTp[:, SH * jj : SH * jj + SH],
                            qT_sb[:, 512 * h2 : 512 * h2 + 512],
                            start=True,
                            stop=True,
                            tile_position=tp,
                        )
                        bass_rust.add_dep_helper(
                            mm.ins,
                            whh_last[0].ins,
                            sync=False,
                            reason="w mm after whh",
                        )
                wa = wap.tile([128, HIDDEN_DIM], F16, name=f"wa{p}", tag="wa")
                wabufs[p] = pw, wa

            def emit_wa_chunk(p, lo, hi, after=None, final=False):
                # tanh over one slice of w, sized to ride an Act gap of the
                # chain (320 cols behind sigma, 192 behind tct).  The
                # scheduler orders by readiness, so pin it behind the chain
                # op with a no-sync edge (same engine, program order is
                # enough).
                pw, wa = wabufs[p]
                ch = nc.scalar.activation(
                    wa[:, lo:hi], pw[:, lo:hi], AF.Tanh, scale=inv_scale
                )
                if after is not None:
                    bass_rust.add_dep_helper(
                        ch.ins, after.ins, sync=False, reason="wa chunk placement"
                    )
                if final:
                    wabufs[p] = wa

            def emit_out(p):
                # transposed: out rows = hidden sub-dim (M=128), col pair =
                # the two steps; 32 pairs accumulate in one PSUM bank
                wa = wabufs.pop(p)
                base = 16 * (p % 32)
                for i in range(8):
                    nc.tensor.matmul(
                        poT[:, base + 2 * i : base + 2 * i + 2],
                        wa[:, 128 * i : 128 * i + 128],
                        xTp_sb[:, 2 * p : 2 * p + 2],
                        start=True,
                        stop=True,
                    )
                if p % 8 == 7:
                    pending_flush.append((p // 32, (p % 32) // 8, None))

            emit_xa(0)
            emit_xa(1)
            for t in range(T):
                j = t % 2  # position within the step pair
                p = t // 2
                if j == 0:
                    h16bufs[p] = hsp.tile(
                        [POS_EM_DIM, 2 * SH], F16, name=f"h16p{p}", tag="h16p"
                    )
                h_prev = (
                    h0[:]
                    if t == 0
                    else (
                        h16bufs[p - 1][:, SH:] if j == 0 else h16bufs[p][:, 0:SH]
                    )
                )
                # ---- gates: W_hh half, on the chain ----
                pg = pgbufs.pop(t)
                for b in range(4):
                    wm = nc.tensor.matmul(
                        pg[:, 64 * b : 64 * b + 64],
                        whhT_sb[:, 128 * b : 128 * b + 128],
                        h_prev,
                        start=False,
                        stop=(b == 3),
                    )
                whh_last[0] = wm
                # ---- merged sigmoid over i,f,2g,o (fp16 out) ----
                sig = work.tile([128, 256], F16, tag="sig")
                sg = nc.scalar.activation(sig[:], pg[:], AF.Sigmoid)
                # attention-tail tanh chunk rides the Act gap here
                if p >= 2:
                    emit_wa_chunk(p - 2, 512 * j, 512 * j + 274, after=sg)
                # ---- c ----  (tanh(g) = 2*sigmoid(2g) - 1)
                t1 = work.tile([128, SH], F16, tag="t1")
                nc.gpsimd.tensor_scalar(
                    t1[:], sig[:, 128:192], 2.0, -1.0, ALU.mult, ALU.add
                )
                nc.gpsimd.tensor_mul(t1[:], t1[:], sig[:, 0:64])
                nc.gpsimd.tensor_mul(cT[:], sig[:, 64:128], cT[:])
                ca = nc.gpsimd.tensor_add(cT[:], cT[:], t1[:])
                if pending_flush:
                    g, hh, _ = pending_flush.pop(0)
                    if hh == 0:
                        so_cur[0] = stg.tile([128, 512], F32, name="so", tag="so")
                    so = so_cur[0]
                    cp = nc.vector.tensor_copy(
                        so[:, 128 * hh : 128 * hh + 128],
                        poT[:, 128 * hh : 128 * hh + 128],
                    )
                    bass_rust.add_dep_helper(
                        cp.ins, ca.ins, sync=False, reason="flush placement"
                    )
                    if hh == 3:
                        nc.sync.dma_start(
                            out=outs[g, :, 384:512], in_=so[:, 384:512]
                        )
                    elif hh == 2:
                        nc.sync.dma_start(
                            out=outs[g, :, 0:384], in_=so[:, 0:384]
                        )
                tct = work.tile([128, SH], F16, tag="tct")
                tc = nc.scalar.activation(tct[:], cT[:], AF.Tanh)
                tct_last[0] = tc
                if p >= 2:
                    emit_wa_chunk(
                        p - 2,
                        512 * j + 274,
                        512 * j + 512,
                        after=tc,
                        final=(j == 1),
                    )
                # ---- h16 = sigma_o * tanh(c), on GPSIMD ----
                nc.gpsimd.tensor_mul(
                    h16bufs[p][:, SH * j : SH * j + SH], sig[:, 192:256], tct[:]
                )
                # ---- pipelined attention tail + next xa prefetch ----
                if j == 1:
                    if p >= 1:
                        emit_kT(p - 1)
                        emit_w_mms(p - 1)
                    if p == T // 2 - 1:
                        # last pair: split the k/w cascade per step-half so
                        # only the odd half trails the final h16
                        kTp_l = work.tile(
                            [MSG_DIM, 2 * SH], F16, name="kTpl", tag="kTp"
                        )
                        pw_l = pwp.tile(
                            [128, HIDDEN_DIM], F32, name="pwl", tag="pw"
                        )
                        wa_l = wap.tile(
                            [128, HIDDEN_DIM], F16, name="wal", tag="wa"
                        )
                        wabufs[p] = pw_l, wa_l
                        w_jj0 = []
                        for half in range(2):
                            pkh = pkp.tile(
                                [MSG_DIM, 2 * SH], F32, name=f"pkl{half}", tag="pk"
                            )
                            sl = slice(SH * half, SH * half + SH)
                            nc.tensor.matmul(
                                pkh[:, sl],
                                wkThi_sb[:],
                                h16bufs[p][:, sl],
                                start=True,
                                stop=False,
                            )
                            k2 = nc.tensor.matmul(
                                pkh[:, sl],
                                wkTlo_sb[:],
                                h16bufs[p][:, sl],
                                start=False,
                                stop=True,
                            )
                            if half == 1:
                                for mm in w_jj0:
                                    bass_rust.add_dep_helper(
                                        mm.ins,
                                        k2.ins,
                                        sync=False,
                                        reason="w-jj0 after kT-b",
                                    )
                            nc.vector.tensor_scalar_add(
                                kTp_l[:, sl], pkh[:, sl], bkc_sb[:]
                            )
                            tp = None if half == 0 else (0, 64)
                            for h2 in range(2):
                                mm = nc.tensor.matmul(
                                    pw_l[
                                        64 * half : 64 * half + 64,
                                        512 * h2 : 512 * h2 + 512,
                                    ],
                                    kTp_l[:, sl],
                                    qT_sb[:, 512 * h2 : 512 * h2 + 512],
                                    start=True,
                                    stop=True,
                                    tile_position=tp,
                                )
                                if half == 0:
                                    w_jj0.append(mm)
                    if p >= 2:
                        emit_out(p - 2)
                    h16bufs.pop(p - 2, None)
                if t + 2 < T:
                    emit_xa(t + 2)
            last = T // 2 - 1  # 127
            emit_wa_chunk(last - 1, 0, 512, after=tct_last[0])
            emit_wa_chunk(last - 1, 512, 1024, final=True)
            emit_out(last - 1)
            emit_wa_chunk(last, 0, 1024, final=True)
            emit_out(last)
            if pending_flush:
                g, hh, _ = pending_flush[0]
                if hh == 0:
                    so_cur[0] = stg.tile([128, 512], F32, name="so", tag="so")
                so = so_cur[0]
                nc.vector.tensor_copy(so[:, 128 * hh : 512], poT[:, 128 * hh : 512])
                if hh <= 2:
                    nc.sync.dma_start(out=outs[g, :, 0:384], in_=so[:, 0:384])
                nc.sync.dma_start(out=outs[g, :, 384:512], in_=so[:, 384:512])
    if split:
        nc.m = _split_multi_waits(nc.m)
    _NC_CACHE[split] = nc
    return nc


def kernel(
    obs,
    prev_act,
    in_shift,
    in_scale,
    pos_embedding,
    W_ih,
    b_ih,
    W_hh,
    b_hh,
    Wq,
    bq,
    Wk,
    bk,
):
    obs = np.asarray(obs, np.float32)
    prev_act = np.asarray(prev_act, np.float32)
    in_shift = np.asarray(in_shift, np.float32)
    in_scale = np.asarray(in_scale, np.float32)
    pos_embedding = np.asarray(pos_embedding, np.float32)
    W_ih = np.asarray(W_ih, np.float32)
    b_ih = np.asarray(b_ih, np.float32)
    W_hh = np.asarray(W_hh, np.float32)
    b_hh = np.asarray(b_hh, np.float32)
    Wq = np.asarray(Wq, np.float32)
    bq = np.asarray(bq, np.float32)
    Wk = np.asarray(Wk, np.float32)
    bk = np.asarray(bk, np.float32)

    x = (obs - in_shift) / (in_scale + 1e-8)  # (T, 512)
    q = pos_embedding @ Wq.T + bq  # (1024, 128)
    qT = np.ascontiguousarray(q.T)  # (128, 1024)

    def blocks(mat_rows):  # reorder gate blocks to [i, f, g, o]
        return np.concatenate([mat_rows[128 * p : 128 * p + 128] for p in _PERM], 0)

    W_ih_r = blocks(W_ih)  # (512, 33)
    W_hh_r = blocks(W_hh)  # (512, 128)
    b_r = blocks((b_ih + b_hh)[:, None])[:, 0]  # (512,)
    # g block (cols 256:384 after reorder) doubled: tanh(g) = 2*sig(2g)-1
    gs = np.ones((512, 1), np.float32)
    gs[256:384] = 2.0
    W_ih_r = W_ih_r * gs
    W_hh_r = W_hh_r * gs
    b_r = b_r * gs[:, 0]

    whhT = np.ascontiguousarray(W_hh_r.T)  # (128, 512)
    wih1T = np.concatenate(
        [W_ih_r[:, 0:1].T, np.ascontiguousarray(W_ih_r[:, 1:33].T), b_r[None, :]], 0
    )  # (34, 512)
    # fp16 hi/lo folded into K: [Whi; Whi; Wlo] x [xhi; xlo; xhi] gives
    # Whi*xhi + Whi*xlo + Wlo*xhi (residual ~2^-22) in one K=102 fp16 MM
    whi = wih1T.astype(np.float16)
    wlo = (wih1T - whi.astype(np.float32)).astype(np.float16)
    wih3T = np.concatenate([whi, whi, wlo], 0)  # (102, 512) fp16
    wkT = np.ascontiguousarray(Wk.T)  # (128, 128) fp32
    wkThi = wkT.astype(np.float16)
    wkTlo = (wkT - wkThi.astype(np.float32)).astype(np.float16)

    nc = _build_nc()
    shared = {
        "whhT": whhT.astype(np.float16),
        "wih3T": np.ascontiguousarray(wih3T),
        "wkThi": wkThi,
        "wkTlo": np.ascontiguousarray(wkTlo),
        "bkc": np.ascontiguousarray(bk[:, None]),
        "qT": qT.astype(np.float16),
    }
    in_maps = []
    for c in range(NCORES):
        xs = x[:, c * SH : (c + 1) * SH]  # (T, 64)
        xa1 = np.empty((34, T * SH), np.float32)
        xa1[0] = xs.reshape(-1)
        xa1[1:33] = np.repeat(prev_act.T, SH, axis=1).reshape(32, T * SH)
        xa1[33] = 1.0
        xahi = xa1.astype(np.float16)
        xalo = (xa1 - xahi.astype(np.float32)).astype(np.float16)
        xa3 = np.concatenate([xahi, xalo, xahi], 0)  # (102, T*SH) fp16
        # block-diagonal paired s columns: col t has s_t in rows [64j, 64j+64)
        # for j = t%2, zeros elsewhere
        xTp = np.zeros((2 * SH, T), np.float16)
        xTp[0:SH, 0::2] = xs.T[:, 0::2]
        xTp[SH : 2 * SH, 1::2] = xs.T[:, 1::2]
        in_maps.append({**shared, "xa3": xa3, "xTp": xTp})

    res = run_bass_kernel_spmd(nc, in_maps, list(range(NCORES)), trace=TRACE[0])
    LAST_RESULTS[0] = res
    total = np.zeros((T, HIDDEN_DIM), np.float32)
    for c in range(NCORES):
        raw = res.results[c]["outs"]  # (T//64, 128, 512)
        # col = (pair%32)*16 + hidden_block*2 + step_in_pair
        total += np.transpose(
            raw.reshape(T // 64, 128, 32, 8, 2), (0, 2, 4, 3, 1)
        ).reshape(T, HIDDEN_DIM)
    return np.tanh(total).astype(np.float32)


# revision 71
# speedup vs baseline: 1.0061x; 1.0005x over previous
"""AttentionNeuronLayer Trainium2 kernel.

Strategy: the obs_dim rows of the LSTM state evolve fully independently
through the whole recurrence (each obs scalar is paired with the full
action vector); only the final `out = w @ s` contracts over obs rows, and
the final tanh is applied after that sum.  So we shard obs_dim 512 -> 64
rows per core across 8 cores with zero collectives: each core runs the
full T=256 recurrence on its 64 rows in a transposed layout
(pos_em/gate/msg dims on partitions, obs rows on the free dim), computes
its slice of w = tanh(q @ k.T / sqrt(d)) and the partial contraction
w.T @ s, and the host sums the 8 partial (T, hidden) results and applies
the final tanh.

The per-step critical path is latency-bound: one serial chain
PE(whh) -> Act(sigma) -> elementwise c-update -> Act(tanh c) ->
h16-mul -> PE per step, each cross-engine hop costing ~100ns of
semaphore visibility, and the Act engine carries the attention tail's
tanh(w) on top of its two chain ops.  In steady state every Act gap is
filled and the step settles at ~1.43us = sigma(398) + w-chunk(412) +
tct(238) + w-chunk(385), Act-saturated.  Scheduling choices:
  * the x/a input half of the gates is accumulated into the step's PSUM
    bank ahead of time as one accumulation group (first xa matmul opens
    it, last whh matmul closes it), so after h16 lands only the four
    27ns W_hh matmuls remain on the chain;
  * one merged 256-wide sigmoid covers i,f,2g,o (g doubled so
    tanh(g) = 2*sigmoid(2g)-1 folds into the c-update);
  * the whole c-update (u = 2*sig2g-1, m = u*i, c = f*c + m) and
    h16 = sigma_o * tct run on the otherwise-idle GPSIMD engine, whose
    elementwise ops carry no SBUF-access bubble; sigma outputs, t1, cT,
    and tct are fp16;
  * the attention tail is pipelined behind the recurrence: fp16-h kT
    matmuls (hi/lo Wk weights), k-bias folded into the DVE PSUM->SBUF
    cast, w matmuls (the second hidden-half scheduling-pinned behind the
    next whh so they never head-of-line block the chain on the in-order
    PE), and tanh(w) split into 274/238-wide chunks pinned into the two
    Act gaps of each step (after sigma and after tct);
  * poT out-accumulation flushes in 128-col quarters through the DVE's
    idle window; input DMAs are chunked and spread across queues so the
    recurrence starts ~3.5us in; a dummy-matmul burst pre-warms the PE
    clock ramp during the DMA wait.
"""

import sys

sys.path.insert(0, "/opt/trn_rl_repo")

import numpy as np

import bass_rust
import concourse.bass as bass
import concourse.tile as tile
from concourse import mybir
from concourse.vector_clock import ScopedClock
from concourse.bass_utils import run_bass_kernel_spmd

OBS_DIM = 512
ACT_DIM = 32
HIDDEN_DIM = 1024
MSG_DIM = 128
POS_EM_DIM = 128
T = 256
NCORES = 8
SH = OBS_DIM // NCORES  # 64 obs rows per core

F32 = mybir.dt.float32
F16 = mybir.dt.float16
AF = mybir.ActivationFunctionType
ALU = mybir.AluOpType

# gate blocks laid out [i, f, g, o]; g row pre-scaled by 2
_PERM = [0, 1, 2, 3]

TRACE = [False]  # test.py flips this for the profiled run
LAST_RESULTS = [None]
INPUT_NAMES = ("whhT", "wih3T", "wkThi", "wkTlo", "bkc", "qT", "xa3", "xTp")


def _patched_drain_and_barrier(self, tick_clock, wait_clock):
    # This walrus build rejects instructions carrying more than one
    # sync-wait command; Tile's tail drain aggregates one wait per live
    # proc.  Re-emit the waits on individual single-wait NOPs instead.
    nc = self.nc
    carrier = nc.sync.nop(nofuse=True)
    wait_clock.add_sem_waits(carrier.ins, ScopedClock({None: tick_clock.global_clock}))
    si = carrier.ins.sync_info
    waits = list(si.on_wait) if si is not None and si.on_wait else []
    if si is not None:
        carrier.ins.sync_info = mybir.SyncInfo(
            on_wait=[], on_update=list(si.on_update or [])
        )
    for w in waits:
        n2 = nc.sync.nop(nofuse=True)
        n2.ins.sync_info = mybir.SyncInfo(on_wait=[w], on_update=[])
    nc.sync.drain()
    nc.all_engine_barrier()
    popped = nc._tile_sem_poison_stack.pop()
    assert popped is self._sem_poison
    nc.clear_and_free_semaphores(list(self.sems.allocated().values()))
    nc.all_engine_barrier()


tile.TileContext._drain_and_barrier = _patched_drain_and_barrier


def _split_multi_waits(module):
    """This walrus build accepts at most one sync-wait command per
    instruction.  Move excess waits onto same-engine NoOps inserted just
    before the instruction — the engine stream is serial, so gating an
    earlier NoOp on the same conditions is equivalent (DMA triggers are
    issued by their engine in program order, so this holds for DMACopy
    too)."""
    import copy as _copy

    counter = [0]
    new_module = _copy.replace(module, functions=[])
    for function in module.functions:
        new_function = _copy.replace(function, blocks=[])
        new_function.set_allocations_from_list(function.allocations)
        for block in function.blocks:
            new_insts = []
            for inst in block.instructions:
                si = inst.sync_info
                waits = list(si.on_wait) if si is not None and si.on_wait else []
                if len(waits) > 1:
                    for w in waits[:-1]:
                        counter[0] += 1
                        nop = mybir.InstNoOp(
                            engine=inst.engine, name=f"I-ws{counter[0]}"
                        )
                        nop.sync_info = mybir.SyncInfo(on_wait=[w], on_update=[])
                        new_insts.append(nop)
                    inst.sync_info = mybir.SyncInfo(
                        on_wait=[waits[-1]], on_update=list(si.on_update or [])
                    )
                new_insts.append(inst)
            new_function.blocks.append(_copy.replace(block, instructions=new_insts))
        new_module.functions.append(new_function)
    return new_module


_NC_CACHE = {}


def _build_nc(split=True):
    if split in _NC_CACHE:
        return _NC_CACHE[split]
    nc = bass.Bass()
    whhT = nc.declare_dram_parameter("whhT", [POS_EM_DIM, 512], F16, isOutput=False)
    wih3T = nc.declare_dram_parameter("wih3T", [102, 512], F16, isOutput=False)
    wkThi = nc.declare_dram_parameter("wkThi", [POS_EM_DIM, MSG_DIM], F16, isOutput=False)
    wkTlo = nc.declare_dram_parameter("wkTlo", [POS_EM_DIM, MSG_DIM], F16, isOutput=False)
    bkc = nc.declare_dram_parameter("bkc", [MSG_DIM, 1], F32, isOutput=False)
    qT = nc.declare_dram_parameter("qT", [MSG_DIM, HIDDEN_DIM], F16, isOutput=False)
    xa3 = nc.declare_dram_parameter("xa3", [102, T * SH], F16, isOutput=False)
    xTp = nc.declare_dram_parameter("xTp", [2 * SH, T], F16, isOutput=False)
    # transposed out accumulation: 64 steps (32 pairs x 8 hidden blocks x
    # N=2) fill one (128, 512) PSUM bank; host decodes the layout
    outs = nc.declare_dram_parameter("outs", [T // 64, 128, 512], F32, isOutput=True)

    inv_scale = 1.0 / float(np.sqrt(np.float32(MSG_DIM)))

    with tile.TileContext(nc) as tc:
        with (
            tc.tile_pool(name="const", bufs=1) as const,
            tc.tile_pool(name="state", bufs=1) as state,
            tc.tile_pool(name="hs", bufs=4) as hsp,
            tc.tile_pool(name="work", bufs=3) as work,
            tc.tile_pool(name="wap", bufs=2) as wap,
            tc.tile_pool(name="stg", bufs=2) as stg,
            tc.tile_pool(name="pg", bufs=2, space="PSUM") as pgp,
            tc.tile_pool(name="pk", bufs=1, space="PSUM") as pkp,
            tc.tile_pool(name="pw", bufs=2, space="PSUM") as pwp,
            tc.tile_pool(name="po", bufs=1, space="PSUM") as pop,
        ):
            whhT_sb = const.tile([POS_EM_DIM, 512], F16)
            wih3T_sb = const.tile([102, 512], F16)
            wkThi_sb = const.tile([POS_EM_DIM, MSG_DIM], F16)
            wkTlo_sb = const.tile([POS_EM_DIM, MSG_DIM], F16)
            bkc_sb = const.tile([MSG_DIM, 1], F32)
            qT_sb = const.tile([MSG_DIM, HIDDEN_DIM], F16)
            xa3_sb = const.tile([102, T * SH], F16)
            xTp_sb = const.tile([2 * SH, T], F16)
            # the recurrence can start once the weights and the first slice
            # of xa3 are resident: front-load those on the SP queue and
            # spread the rest (xa3 streams far ahead of consumption) across
            # the other engines' DMA queues so nothing serializes behind
            # the 3.2MB xa3 transfer
            XQ = T * SH // 4
            # all bulk DMAs ride the SP queue (SP is otherwise idle until
            # the first out-flush at ~100us); only qT uses the Act queue
            # (lands well before the first w matmuls).  Pool/DVE must stay
            # clear: DMA transfers occupy the issuing engine in the cost
            # model, and the chain needs them from ~7us on.
            nc.sync.dma_start(out=whhT_sb[:], in_=whhT[:])
            nc.sync.dma_start(out=wih3T_sb[:], in_=wih3T[:])
            nc.sync.dma_start(out=xa3_sb[:, 0 : XQ // 2], in_=xa3[:, 0 : XQ // 2])
            nc.scalar.dma_start(out=qT_sb[:], in_=qT[:])
            nc.sync.dma_start(out=xa3_sb[:, XQ // 2 : XQ], in_=xa3[:, XQ // 2 : XQ])
            nc.sync.dma_start(out=wkThi_sb[:], in_=wkThi[:])
            nc.sync.dma_start(out=wkTlo_sb[:], in_=wkTlo[:])
            nc.sync.dma_start(out=bkc_sb[:], in_=bkc[:])
            nc.sync.dma_start(out=xTp_sb[:], in_=xTp[:])
            nc.sync.dma_start(out=xa3_sb[:, XQ : 2 * XQ], in_=xa3[:, XQ : 2 * XQ])
            nc.sync.dma_start(
                out=xa3_sb[:, 2 * XQ : 3 * XQ], in_=xa3[:, 2 * XQ : 3 * XQ]
            )
            nc.sync.dma_start(
                out=xa3_sb[:, 3 * XQ : 4 * XQ], in_=xa3[:, 3 * XQ : 4 * XQ]
            )

            cT = state.tile([POS_EM_DIM, SH], F16)
            h0 = const.tile([POS_EM_DIM, SH], F16)
            nc.vector.memset(cT[:], 0.0)
            nc.vector.memset(h0[:], 0.0)
            # warm the sigmoid/tanh act table while the input DMAs stream
            scr = const.tile([128, 1], F32)
            nc.scalar.activation(scr[:], h0[:, 0:1], AF.Sigmoid)

            poT = pop.tile([128, 512], F32)
            nc.vector.memset(poT[:], 0.0)

            # PE p-state pre-warm: ~3us of dummy matmuls while the input
            # DMAs stream, so the real recurrence starts at full clock
            # (the tensor engine ramps 0.65 -> 2.4 GHz over 3us of use)
            pwm = pkp.tile([MSG_DIM, 2 * SH], F32, name="pwm", tag="pk")
            for _ in range(46):
                nc.tensor.matmul(pwm[0:64, 0:64], h0[:, 0:64], h0[:, 0:64], start=True, stop=True)

            h16bufs = {}
            kbufs = {}
            wabufs = {}
            pgbufs = {}
            pending_flush = []
            whh_last = [None]
            tct_last = [None]
            so_cur = [None]

            def emit_xa(t):
                # x/a input half of step t's gates: no h dependence, runs in
                # PE idle time ahead of the chain.  One PSUM accumulation
                # group covers the whole step (zero regions are
                # bank-granular, so only one group may be open per bank):
                # the first xa matmul opens it, the last whh matmul closes
                # it.  Full-bank tile so nothing else shares the region.
                pg = pgp.tile([128, 512], F32, name=f"pg{t}", tag="pg")[:, 0:256]
                pgbufs[t] = pg
                for b in range(4):
                    nc.tensor.matmul(
                        pg[:, 64 * b : 64 * b + 64],
                        wih3T_sb[:, 128 * b : 128 * b + 128],
                        xa3_sb[:, SH * t : SH * t + SH],
                        start=(b == 0),
                        stop=False,
                    )

            def emit_kT(p):
                # k = Wk @ h for the step pair, fp16 h with hi/lo weights
                pk = pkp.tile([MSG_DIM, 2 * SH], F32, tag="pk")
                nc.tensor.matmul(
                    pk[:], wkThi_sb[:], h16bufs[p][:], start=True, stop=False
                )
                nc.tensor.matmul(
                    pk[:], wkTlo_sb[:], h16bufs[p][:], start=False, stop=True
                )
                # PSUM->SBUF fp16 cast with the k-bias folded in; GPSIMD
                # cannot read PSUM, so this rides the DVE's idle window
                # between the c-chain and the next step
                kTp = work.tile([MSG_DIM, 2 * SH], F16, name=f"kTp{p}", tag="kTp")
                nc.vector.tensor_scalar_add(kTp[:], pk[:], bkc_sb[:])
                kbufs[p] = kTp

            def emit_w_mms(p):
                # rows 0-63 = even step of the pair, 64-127 = odd step.
                # The second hidden-half's matmuls get scheduling edges
                # behind the NEXT step's whh matmuls (pending_wdep) so the
                # 213ns w matmuls never head-of-line block the chain on
                # the in-order PE.
                kTp = kbufs.pop(p)
                pw = pwp.tile([128, HIDDEN_DIM], F32, name=f"pw{p}", tag="pw")
                for h2 in range(2):
                    for jj in range(2):
                        tp = None if jj == 0 else (0, 64)
                        mm = nc.tensor.matmul(
                            pw[64 * jj : 64 * jj + 64, 512 * h2 : 512 * h2 + 512],
                            kTp[:, SH * jj : SH * jj + SH],
                            qT_sb[:, 512 * h2 : 512 * h2 + 512],
                            start=True,
                            stop=True,
                            tile_position=tp,
                        )
                        bass_rust.add_dep_helper(
                            mm.ins,
                            whh_last[0].ins,
                            sync=False,
                            reason="w mm after whh",
                        )
                wa = wap.tile([128, HIDDEN_DIM], F16, name=f"wa{p}", tag="wa")
                wabufs[p] = pw, wa

            def emit_wa_chunk(p, lo, hi, after=None, final=False):
                # tanh over one slice of w, sized to ride an Act gap of the
                # chain (320 cols behind sigma, 192 behind tct).  The
                # scheduler orders by readiness, so pin it behind the chain
                # op with a no-sync edge (same engine, program order is
                # enough).
                pw, wa = wabufs[p]
                ch = nc.scalar.activation(
                    wa[:, lo:hi], pw[:, lo:hi], AF.Tanh, scale=inv_scale
                )
                if after is not None:
                    bass_rust.add_dep_helper(
                        ch.ins, after.ins, sync=False, reason="wa chunk placement"
                    )
                if final:
                    wabufs[p] = wa

            def emit_out(p):
                # transposed: out rows = hidden sub-dim (M=128), col pair =
                # the two steps; 32 pairs accumulate in one PSUM bank
                wa = wabufs.pop(p)
                base = 16 * (p % 32)
                for i in range(8):
                    nc.tensor.matmul(
                        poT[:, base + 2 * i : base + 2 * i + 2],
                        wa[:, 128 * i : 128 * i + 128],
                        xTp_sb[:, 2 * p : 2 * p + 2],
                        start=True,
                        stop=True,
                    )
                if p % 8 == 7:
                    pending_flush.append((p // 32, (p % 32) // 8, None))

            emit_xa(0)
            emit_xa(1)
            for t in range(T):
                j = t % 2  # position within the step pair
                p = t // 2
                if j == 0:
                    h16bufs[p] = hsp.tile(
                        [POS_EM_DIM, 2 * SH], F16, name=f"h16p{p}", tag="h16p"
                    )
                h_prev = (
                    h0[:]
                    if t == 0
                    else (
                        h16bufs[p - 1][:, SH:] if j == 0 else h16bufs[p][:, 0:SH]
                    )
                )
                # ---- gates: W_hh half, on the chain ----
                pg = pgbufs.pop(t)
                for b in range(4):
                    wm = nc.tensor.matmul(
                        pg[:, 64 * b : 64 * b + 64],
                        whhT_sb[:, 128 * b : 128 * b + 128],
                        h_prev,
                        start=False,
                        stop=(b == 3),
                    )
                whh_last[0] = wm
                # ---- merged sigmoid over i,f,2g,o (fp16 out) ----
                sig = work.tile([128, 256], F16, tag="sig")
                sg = nc.scalar.activation(sig[:], pg[:], AF.Sigmoid)
                # attention-tail tanh chunk rides the Act gap here
                if p >= 2:
                    emit_wa_chunk(p - 2, 512 * j, 512 * j + 274, after=sg)
                # ---- c ----  (tanh(g) = 2*sigmoid(2g) - 1)
                t1 = work.tile([128, SH], F16, tag="t1")
                nc.gpsimd.tensor_scalar(
                    t1[:], sig[:, 128:192], 2.0, -1.0, ALU.mult, ALU.add
                )
                nc.gpsimd.tensor_mul(t1[:], t1[:], sig[:, 0:64])
                nc.gpsimd.tensor_mul(cT[:], sig[:, 64:128], cT[:])
                ca = nc.gpsimd.tensor_add(cT[:], cT[:], t1[:])
                if pending_flush:
                    g, hh, _ = pending_flush.pop(0)
                    if hh == 0:
                        so_cur[0] = stg.tile([128, 512], F32, name="so", tag="so")
                    so = so_cur[0]
                    cp = nc.vector.tensor_copy(
                        so[:, 128 * hh : 128 * hh + 128],
                        poT[:, 128 * hh : 128 * hh + 128],
                    )
                    bass_rust.add_dep_helper(
                        cp.ins, ca.ins, sync=False, reason="flush placement"
                    )
                    if hh == 3:
                        nc.sync.dma_start(
                            out=outs[g, :, 384:512], in_=so[:, 384:512]
                        )
                    elif hh == 2:
                        nc.sync.dma_start(
                            out=outs[g, :, 0:384], in_=so[:, 0:384]
                        )
                tct = work.tile([128, SH], F16, tag="tct")
                tc = nc.scalar.activation(tct[:], cT[:], AF.Tanh)
                tct_last[0] = tc
                if p >= 2:
                    emit_wa_chunk(
                        p - 2,
                        512 * j + 274,
                        512 * j + 512,
                        after=tc,
                        final=(j == 1),
                    )
                # ---- h16 = sigma_o * tanh(c), on GPSIMD ----
                nc.gpsimd.tensor_mul(
                    h16bufs[p][:, SH * j : SH * j + SH], sig[:, 192:256], tct[:]
                )
                # ---- pipelined attention tail + next xa prefetch ----
                if j == 1:
                    if p >= 1:
                        emit_kT(p - 1)
                        emit_w_mms(p - 1)
                    if p == T // 2 - 1:
                        # last pair: split the k/w cascade per step-half so
                        # only the odd half trails the final h16
                        kTp_l = work.tile(
                            [MSG_DIM, 2 * SH], F16, name="kTpl", tag="kTp"
                        )
                        pw_l = pwp.tile(
                            [128, HIDDEN_DIM], F32, name="pwl", tag="pw"
                        )
                        wa_l = wap.tile(
                            [128, HIDDEN_DIM], F16, name="wal", tag="wa"
                        )
                        wabufs[p] = pw_l, wa_l
                        w_jj0 = []
                        for half in range(2):
                            pkh = pkp.tile(
                                [MSG_DIM, 2 * SH], F32, name=f"pkl{half}", tag="pk"
                            )
                            sl = slice(SH * half, SH * half + SH)
                            nc.tensor.matmul(
                                pkh[:, sl],
                                wkThi_sb[:],
                                h16bufs[p][:, sl],
                                start=True,
                                stop=False,
                            )
                            k2 = nc.tensor.matmul(
                                pkh[:, sl],
                                wkTlo_sb[:],
                                h16bufs[p][:, sl],
                                start=False,
                                stop=True,
                            )
                            if half == 1:
                                for mm in w_jj0:
                                    bass_rust.add_dep_helper(
                                        mm.ins,
                                        k2.ins,
                                        sync=False,
                                        reason="w-jj0 after kT-b",
                                    )
                            nc.vector.tensor_scalar_add(
                                kTp_l[:, sl], pkh[:, sl], bkc_sb[:]
                            )
                            tp = None if half == 0 else (0, 64)
                            for h2 in range(2):
                                mm = nc.tensor.matmul(
                                    pw_l[
                                        64 * half : 64 * half + 64,
                                        512 * h2 : 512 * h2 + 512,
                                    ],
                                    kTp_l[:, sl],
                                    qT_sb[:, 512 * h2 : 512 * h2 + 512],
                                    start=True,
                                    stop=True,
                                    tile_position=tp,
                                )
                                if half == 0:
                                    w_jj0.append(mm)
                    if p >= 2:
                        emit_out(p - 2)
                    h16bufs.pop(p - 2, None)
                if t + 2 < T:
                    emit_xa(t + 2)
            last = T // 2 - 1  # 127
            emit_wa_chunk(last - 1, 0, 1024, after=tct_last[0], final=True)
            emit_out(last - 1)
            emit_wa_chunk(last, 0, 1024, final=True)
            emit_out(last)
            if pending_flush:
                g, hh, _ = pending_flush[0]
                if hh == 0:
                    so_cur[0] = stg.tile([128, 512], F32, name="so", tag="so")
                so = so_cur[0]
                nc.vector.tensor_copy(so[:, 128 * hh : 512], poT[:, 128 * hh : 512])
                if hh <= 2:
                    nc.sync.dma_start(out=outs[g, :, 0:384], in_=so[:, 0:384])
                nc.sync.dma_start(out=outs[g, :, 384:512], in_=so[:, 384:512])
    if split:
        nc.m = _split_multi_waits(nc.m)
    _NC_CACHE[split] = nc
    return nc


def kernel(
    obs,
    prev_act,
    in_shift,
    in_scale,
    pos_embedding,
    W_ih,
    b_ih,
    W_hh,
    b_hh,
    Wq,
    bq,
    Wk,
    bk,
):
    obs = np.asarray(obs, np.float32)
    prev_act = np.asarray(prev_act, np.float32)
    in_shift = np.asarray(in_shift, np.float32)
    in_scale = np.asarray(in_scale, np.float32)
    pos_embedding = np.asarray(pos_embedding, np.float32)
    W_ih = np.asarray(W_ih, np.float32)
    b_ih = np.asarray(b_ih, np.float32)
    W_hh = np.asarray(W_hh, np.float32)
    b_hh = np.asarray(b_hh, np.float32)
    Wq = np.asarray(Wq, np.float32)
    bq = np.asarray(bq, np.float32)
    Wk = np.asarray(Wk, np.float32)
    bk = np.asarray(bk, np.float32)

    x = (obs - in_shift) / (in_scale + 1e-8)  # (T, 512)
    q = pos_embedding @ Wq.T + bq  # (1024, 128)
    qT = np.ascontiguousarray(q.T)  # (128, 1024)

    def blocks(mat_rows):  # reorder gate blocks to [i, f, g, o]
        return np.concatenate([mat_rows[128 * p : 128 * p + 128] for p in _PERM], 0)

    W_ih_r = blocks(W_ih)  # (512, 33)
    W_hh_r = blocks(W_hh)  # (512, 128)
    b_r = blocks((b_ih + b_hh)[:, None])[:, 0]  # (512,)
    # g block (cols 256:384 after reorder) doubled: tanh(g) = 2*sig(2g)-1
    gs = np.ones((512, 1), np.float32)
    gs[256:384] = 2.0
    W_ih_r = W_ih_r * gs
    W_hh_r = W_hh_r * gs
    b_r = b_r * gs[:, 0]

    whhT = np.ascontiguousarray(W_hh_r.T)  # (128, 512)
    wih1T = np.concatenate(
        [W_ih_r[:, 0:1].T, np.ascontiguousarray(W_ih_r[:, 1:33].T), b_r[None, :]], 0
    )  # (34, 512)
    # fp16 hi/lo folded into K: [Whi; Whi; Wlo] x [xhi; xlo; xhi] gives
    # Whi*xhi + Whi*xlo + Wlo*xhi (residual ~2^-22) in one K=102 fp16 MM
    whi = wih1T.astype(np.float16)
    wlo = (wih1T - whi.astype(np.float32)).astype(np.float16)
    wih3T = np.concatenate([whi, whi, wlo], 0)  # (102, 512) fp16
    wkT = np.ascontiguousarray(Wk.T)  # (128, 128) fp32
    wkThi = wkT.astype(np.float16)
    wkTlo = (wkT - wkThi.astype(np.float32)).astype(np.float16)

    nc = _build_nc()
    shared = {
        "whhT": whhT.astype(np.float16),
        "wih3T": np.ascontiguousarray(wih3T),
        "wkThi": wkThi,
        "wkTlo": np.ascontiguousarray(wkTlo),
        "bkc": np.ascontiguousarray(bk[:, None]),
        "qT": qT.astype(np.float16),
    }
    in_maps = []
    for c in range(NCORES):
        xs = x[:, c * SH : (c + 1) * SH]  # (T, 64)
        xa1 = np.empty((34, T * SH), np.float32)
        xa1[0] = xs.reshape(-1)
        xa1[1:33] = np.repeat(prev_act.T, SH, axis=1).reshape(32, T * SH)
        xa1[33] = 1.0
        xahi = xa1.astype(np.float16)
        xalo = (xa1 - xahi.astype(np.float32)).astype(np.float16)
        xa3 = np.concatenate([xahi, xalo, xahi], 0)  # (102, T*SH) fp16
        # block-diagonal paired s columns: col t has s_t in rows [64j, 64j+64)
        # for j = t%2, zeros elsewhere
        xTp = np.zeros((2 * SH, T), np.float16)
        xTp[0:SH, 0::2] = xs.T[:, 0::2]
        xTp[SH : 2 * SH, 1::2] = xs.T[:, 1::2]
        in_maps.append({**shared, "xa3": xa3, "xTp": xTp})

    res = run_bass_kernel_spmd(nc, in_maps, list(range(NCORES)), trace=TRACE[0])
    LAST_RESULTS[0] = res
    total = np.zeros((T, HIDDEN_DIM), np.float32)
    for c in range(NCORES):
        raw = res.results[c]["outs"]  # (T//64, 128, 512)
        # col = (pair%32)*16 + hidden_block*2 + step_in_pair
        total += np.transpose(
            raw.reshape(T // 64, 128, 32, 8, 2), (0, 2, 4, 3, 1)
        ).reshape(T, HIDDEN_DIM)
    return np.tanh(total).astype(np.float32)
